# revision 39
# baseline (speedup 1.0000x reference)
"""Trainium2 Bass kernel for nn_BasicBlock (EfficientViT-style block).

Data-parallel over 8 NeuronCores: batch 64 -> 8 images/core.
Per-core program: dw0 -> MLP0 -> cascaded window attention -> proj -> dw1 -> MLP1.
"""
import itertools
import functools
import numpy as np
import ml_dtypes

import concourse.bass as bass
import concourse.mybir as mybir
import concourse.tile as tile
from concourse import bacc
from concourse import bass_utils

f32 = mybir.dt.float32
f32r = mybir.dt.float32r
bf16 = mybir.dt.bfloat16
AO = mybir.AluOpType
AF = mybir.ActivationFunctionType

ED, KD, NH, AR = 512, 16, 8, 4
D = AR * KD            # 64
DH = D * NH            # 512
RES, WS = 28, 7
SCALE = KD ** -0.5
KS = [7, 5, 3, 3, 3, 3, 3, 3]
NI = 8                 # images per core
NCORES = 8
POS = RES * RES        # 784
NW = 16                # windows per image
WN = WS * WS           # 49


def _bias_idx(ws):
    pts = list(itertools.product(range(ws), range(ws)))
    offs, idxs = {}, []
    for p1 in pts:
        for p2 in pts:
            o = (abs(p1[0] - p2[0]), abs(p1[1] - p2[1]))
            if o not in offs:
                offs[o] = len(offs)
            idxs.append(offs[o])
    return np.array(idxs, dtype=np.int32).reshape(ws * ws, ws * ws), len(offs)


BIAS_IDX, N_OFFS = _bias_idx(WS)


# ---------------------------------------------------------------------------
# program builder
# ---------------------------------------------------------------------------

def _dw_taps(k):
    return [(dy, dx) for dy in range(k) for dx in range(k)]


def _dw_taps3_sorted():
    """3x3 taps with the full-coverage center tap (1,1) first."""
    return sorted(enumerate(_dw_taps(3)),
                  key=lambda e: (e[1][0] != 1, e[1][1] != 1))


def cell_ap(tile_ap, r0, c0, cw, nb=4):
    """[nb, 7, 7] strided view of `nb` conv cells at rows r0.., cols c0+cw*b."""
    base = tile_ap[:, r0:r0 + 1, c0:c0 + 1]
    part = base.ap[0]
    return bass.AP(base.tensor, base.offset,
                   [[part[0], part[1]], [cw, nb],
                    [tile_ap.shape[2], 7], [1, 7]])


def build_program():
    nc = bacc.Bacc("TRN2", target_bir_lowering=False, debug=False,
                   enable_asserts=False, num_devices=NCORES)

    dt_in = {}

    def din(name, shape, dt=f32):
        t = nc.dram_tensor(name, list(shape), dt, kind="ExternalInput")
        dt_in[name] = t
        return t.ap()

    x_d = din("x", [NI, ED, POS])
    dw0w_d = din("dw0w", [4, 128, 9])
    dw0b_d = din("dw0b", [4, 128])
    w1T0_d = din("w1T0", [ED, 2 * ED], bf16)
    b1f0_d = din("b1f0", [2 * ED], bf16)
    w2T0_d = din("w2T0", [2 * ED, ED], bf16)
    b2f0_d = din("b2f0", [ED])
    wkqT_d = din("wkqT", [NH, D, 2 * KD], bf16)
    bkq_d = din("bkq", [NH, 2 * KD])
    wvT_d = din("wvT", [NH, D, D], bf16)
    bv_d = din("bv", [NH, D])
    dwqw_d = din("dwqw", [NH, 128, 49])
    dwqb_d = din("dwqb", [NH, 128])
    ab_d = din("ab", [NH, WN, 8 * WN])
    projT_d = din("projT", [DH, ED], bf16)
    projb_d = din("projb", [ED])
    yb_d = din("yb", [ED])
    dw1w_d = din("dw1w", [4, 128, 9])
    dw1b_d = din("dw1b", [4, 128])
    w1T1_d = din("w1T1", [ED, 2 * ED], bf16)
    b1f1_d = din("b1f1", [2 * ED], bf16)
    w2T1_d = din("w2T1", [2 * ED, ED], bf16)
    b2f1_d = din("b2f1", [ED])
    ident_d = din("ident", [128, 128], bf16)

    out_d = nc.dram_tensor("out", [NI, ED, POS], f32, kind="ExternalOutput").ap()

    with tile.TileContext(nc) as tc:
        _body(tc, nc, x_d, dw0w_d, dw0b_d, w1T0_d, b1f0_d, w2T0_d, b2f0_d,
              wkqT_d, bkq_d, wvT_d, bv_d, dwqw_d, dwqb_d, ab_d,
              projT_d, projb_d, yb_d, dw1w_d, dw1b_d,
              w1T1_d, b1f1_d, w2T1_d, b2f1_d, out_d, ident_d)

    nc.compile()
    return nc


def _dwconv_block(tc, nc, pads, accs, src_getter, wsb, bsb, dst_writer,
                  pad_dt=bf16):
    for c in range(4):
        for img in range(NI):
            pad = pads.tile([128, 30, 30], pad_dt, tag="dwpad",
                            name=f"pad_{c}_{img}")
            nc.gpsimd.memset(pad, 0.0)
            src_getter(c, img, pad)
            acc = accs.tile([128, 28, 28], bf16, tag="dwacc",
                            name=f"acc_{c}_{img}")
            first = True
            for t, (dy, dx) in enumerate(_dw_taps(3)):
                srcap = pad[:, dy:dy + 28, dx:dx + 28]
                if first:
                    nc.vector.tensor_scalar(acc[:], srcap, wsb[c][:, t:t + 1],
                                            bsb[c][:, 0:1], AO.mult, AO.add)
                    first = False
                else:
                    nc.vector.scalar_tensor_tensor(acc[:], srcap, wsb[c][:, t:t + 1],
                                                   acc[:], AO.mult, AO.add)
            dst_writer(c, img, acc, pad)


def _body(tc, nc, x_d, dw0w_d, dw0b_d, w1T0_d, b1f0_d, w2T0_d, b2f0_d,
          wkqT_d, bkq_d, wvT_d, bv_d, dwqw_d, dwqb_d, ab_d,
          projT_d, projb_d, yb_d, dw1w_d, dw1b_d,
          w1T1_d, b1f1_d, w2T1_d, b2f1_d, out_d, ident_d):
    ctx_pools = []

    # DRAM intermediate: x2 in window-major layout [4, 128, NI, 784]
    dram_cm = tc.tile_pool(name="dram", bufs=1, space="DRAM")
    dram = dram_cm.__enter__()
    # Window index convention: window W = 4*b + a for spatial cell (a, b).
    # x2 lives in SBUF as bf16 window-major for the whole kernel.
    x2wm_cm = tc.tile_pool(name="x2wm", bufs=1)
    x2wm_p = x2wm_cm.__enter__()
    x2wm = [x2wm_p.tile([128, NI, POS], bf16, tag=f"x2wm{c}",
                        name=f"x2wm{c}") for c in range(4)]

    misc_cm = tc.tile_pool(name="misc", bufs=1)
    misc = misc_cm.__enter__()
    ident_sb = misc.tile([128, 128], bf16, tag="ident")
    nc.sync.dma_start(out=ident_sb, in_=ident_d)

    # ---------------- persistent pools -------------------------------------
    xp_cm = tc.tile_pool(name="xp", bufs=1)
    xp = xp_cm.__enter__()
    x_sb = []
    for c in range(4):
        t = xp.tile([128, NI, 28, 28], f32, tag=f"x{c}")
        x_sb.append(t)
        for img in range(NI):
            nc.sync.dma_start(out=t[:, img], in_=x_d[img, 128 * c:128 * (c + 1), :]
                              .rearrange("p (h w) -> p h w", h=28))

    # -------- P1+P2: dw0 + MLP0 + window-major x2 store, pipelined ---------
    with tc.tile_pool(name="dwk0", bufs=1) as dwk, \
         tc.tile_pool(name="m0w", bufs=1) as wp, \
         tc.tile_pool(name="dwp0", bufs=2) as padp, \
         tc.tile_pool(name="dwa0", bufs=2) as accp, \
         tc.tile_pool(name="x1b", bufs=2) as x1bp, \
         tc.tile_pool(name="m0h", bufs=1) as hp, \
         tc.tile_pool(name="m0r", bufs=3) as rp, \
         tc.tile_pool(name="m0z", bufs=3) as zp, \
         tc.tile_pool(name="wms", bufs=2) as wmstp, \
         tc.tile_pool(name="m0ps", bufs=4, space="PSUM") as psp, \
         tc.tile_pool(name="m0po", bufs=2, space="PSUM") as pop:
        w0sb, b0sb = [], []
        for c in range(4):
            w = dwk.tile([128, 9], f32, tag=f"dw0w{c}")
            nc.sync.dma_start(out=w, in_=dw0w_d[c])
            b = dwk.tile([128, 1], f32, tag=f"dw0b{c}")
            nc.sync.dma_start(out=b, in_=dw0b_d[c].unsqueeze(1))
            w0sb.append(w)
            b0sb.append(b)
        w1sb = []
        for k in range(4):
            w = wp.tile([128, 2 * ED], bf16, tag=f"w1_{k}")
            nc.sync.dma_start(out=w, in_=w1T0_d[128 * k:128 * (k + 1), :])
            w1sb.append(w)
        w2sb = []
        for k in range(8):
            w = wp.tile([128, ED], bf16, tag=f"w2_{k}")
            nc.sync.dma_start(out=w, in_=w2T0_d[128 * k:128 * (k + 1), :])
            w2sb.append(w)
        b1row = wp.tile([1, 2 * ED], bf16, tag="b1row")
        nc.sync.dma_start(out=b1row, in_=b1f0_d.unsqueeze(0))
        ones392 = wp.tile([1, 392], bf16, tag="ones392")
        nc.vector.memset(ones392, 1.0)
        acth = wp.tile([128, 1], f32, tag="acth")
        nc.vector.memset(acth, 0.5)
        acts = wp.tile([128, 1], f32, tag="acts")
        nc.vector.memset(acts, 1.0 / 6.0)
        b2sb = []
        for m in range(4):
            b = wp.tile([128, 1], f32, tag=f"b2_{m}")
            nc.sync.dma_start(out=b, in_=b2f0_d[128 * m:128 * (m + 1)].unsqueeze(1))
            b2sb.append(b)

        for j in range(4):
            # dw0: taps read x_sb f32 directly with edge clipping
            for c in range(4):
                for i01 in range(2):
                    im = 2 * j + i01
                    xim = x_sb[c][:, im]
                    acc = accp.tile([128, 28, 28], f32, tag="acc",
                                    name=f"acc{c}_{im}")
                    first = True
                    for t, (dy, dx) in _dw_taps3_sorted():
                        r0, r1 = max(0, dy - 1), 28 + min(0, dy - 1)
                        c0, c1 = max(0, dx - 1), 28 + min(0, dx - 1)
                        src_ = xim[:, r0:r1, c0:c1]
                        dst = acc[:, max(0, 1 - dy):max(0, 1 - dy) + (r1 - r0),
                                  max(0, 1 - dx):max(0, 1 - dx) + (c1 - c0)]
                        if first:
                            # acc = w_center*x + x  (residual folded in)
                            nc.vector.scalar_tensor_tensor(
                                dst, src_, w0sb[c][:, t:t + 1], xim,
                                AO.mult, AO.add)
                            first = False
                        else:
                            nc.vector.scalar_tensor_tensor(
                                dst, src_, w0sb[c][:, t:t + 1], dst,
                                AO.mult, AO.add)
                    # x1 = acc + b0  -> in place (x already folded into acc)
                    nc.vector.scalar_tensor_tensor(
                        xim, acc[:], b0sb[c][:, 0:1], acc[:], AO.add, AO.bypass)
            for i01 in range(2):
                img = 2 * j + i01
                x1b = []
                for c in range(4):
                    t = x1bp.tile([128, POS], bf16, tag=f"x1b{c}",
                                  name=f"x1b{c}_{img}")
                    nc.vector.tensor_copy(
                        t[:], x_sb[c][:, img].rearrange("p h w -> p (h w)"))
                    x1b.append(t)
                hs = [hp.tile([128, POS], bf16, tag=f"h{m}", name=f"h{m}_{img}")
                      for m in range(8)]
                for m in range(8):
                    for n2 in range(2):
                        ph = psp.tile([128, 392], f32, tag="ph")
                        for k in range(4):
                            nc.tensor.matmul(
                                ph[:], w1sb[k][:, 128 * m:128 * (m + 1)],
                                x1b[k][:, 392 * n2:392 * (n2 + 1)],
                                start=(k == 0), stop=False)
                        nc.tensor.matmul(
                            ph[:], b1row[:, 128 * m:128 * (m + 1)],
                            ones392[:], start=False, stop=True)
                        r = rp.tile([128, 392], bf16, tag="relu")
                        nc.scalar.activation(r[:], ph[:], AF.Relu,
                                             scale=acts[:, 0:1], bias=acth[:, 0:1])
                        nc.vector.scalar_tensor_tensor(
                            hs[m][:, 392 * n2:392 * (n2 + 1)], r[:], 1.0,
                            ph[:], AO.min, AO.mult)
                for mo in range(4):
                    x2v = x_sb[mo][:, img].rearrange("p h w -> p (h w)")
                    for n2 in range(2):
                        po = pop.tile([128, 392], f32, tag="po")
                        for k in range(8):
                            nc.tensor.matmul(
                                po[:], w2sb[k][:, 128 * mo:128 * (mo + 1)],
                                hs[k][:, 392 * n2:392 * (n2 + 1)],
                                start=(k == 0), stop=False)
                        nc.tensor.matmul(
                            po[:], ident_sb[:],
                            x1b[mo][:, 392 * n2:392 * (n2 + 1)],
                            start=False, stop=True)
                        nc.scalar.activation(
                            x2v[:, 392 * n2:392 * (n2 + 1)], po[:],
                            AF.Identity, bias=b2sb[mo][:, 0:1])
                    stb = wmstp.tile([128, POS], bf16, tag="stb",
                                     name=f"stb{mo}_{img}")
                    nc.vector.tensor_copy(stb[:], x2v)
                    sbv = stb[:].rearrange("p (a h b w) -> p a h b w",
                                           a=4, h=7, b=4)
                    for b in range(4):
                        nc.gpsimd.tensor_copy(
                            x2wm[mo][:, img, 196 * b:196 * (b + 1)]
                            .rearrange("p (a h w) -> p a h w", a=4, h=7),
                            sbv[:, :, :, b, :])

    xp_cm.__exit__(None, None, None)

    # ---------------- P3: cascaded attention -------------------------------
    # Packing: image i -> partition block 64*(i%2); col block i//2 (sp, vt, ein)
    #          k/q' : image i -> partition block 32*(i%4); col block i//4
    y_dram = dram.tile([4, 128, NI, POS], bf16, name="y_dram")

    atw_cm = tc.tile_pool(name="atw", bufs=1)
    atw = atw_cm.__enter__()
    wkq_sb, bkq_sb, wv_sb, bv_sb, dq_w, dq_b, ab_sb = [], [], [], [], [], [], []
    for h in range(NH):
        t = atw.tile([128, 2 * KD], bf16, tag=f"wkq{h}", name=f"wkq{h}")
        nc.sync.dma_start(out=t[0:64, :], in_=wkqT_d[h])
        nc.sync.dma_start(out=t[64:128, :], in_=wkqT_d[h])
        wkq_sb.append(t)
        t = atw.tile([128, 1], f32, tag=f"bkq{h}", name=f"bkq{h}")
        nc.sync.dma_start(out=t[0:32, :], in_=bkq_d[h].unsqueeze(1))
        nc.sync.dma_start(out=t[64:96, :], in_=bkq_d[h].unsqueeze(1))
        bkq_sb.append(t)
        t = atw.tile([128, D], bf16, tag=f"wv{h}", name=f"wv{h}")
        nc.sync.dma_start(out=t[0:64, :], in_=wvT_d[h])
        nc.sync.dma_start(out=t[64:128, :], in_=wvT_d[h])
        wv_sb.append(t)
        t = atw.tile([128, 1], f32, tag=f"bv{h}", name=f"bv{h}")
        nc.sync.dma_start(out=t[0:64, :], in_=bv_d[h].unsqueeze(1))
        nc.sync.dma_start(out=t[64:128, :], in_=bv_d[h].unsqueeze(1))
        bv_sb.append(t)
        t = atw.tile([128, 49], f32, tag=f"dqw{h}", name=f"dqw{h}")
        nc.sync.dma_start(out=t, in_=dwqw_d[h])
        dq_w.append(t)
        t = atw.tile([128, 1], f32, tag=f"dqb{h}", name=f"dqb{h}")
        nc.sync.dma_start(out=t, in_=dwqb_d[h].unsqueeze(1))
        dq_b.append(t)
        t = atw.tile([128, 392], f32, tag=f"ab{h}", name=f"ab{h}")
        nc.vector.memset(t[:], 0.0)
        nc.sync.dma_start(out=t[0:49, :], in_=ab_d[h])
        nc.sync.dma_start(out=t[64:113, :], in_=ab_d[h])
        ab_sb.append(t)
    # softmax helpers for pair-batched layout (imgs 2j/2j+1 at rows 0/64)
    ones2 = atw.tile([128, 2], bf16, tag="ones2", name="ones2")
    nc.vector.memset(ones2, 0.0)
    nc.vector.memset(ones2[0:49, 0:1], 1.0)
    nc.vector.memset(ones2[64:113, 1:2], 1.0)
    sel2 = atw.tile([2, 128], f32, tag="sel2", name="sel2")
    nc.vector.memset(sel2[0:1, :], 0.0)
    nc.vector.memset(sel2[0:1, 0:64], 1.0)
    sel2r1 = atw.tile([1, 128], f32, tag="sel2r1", name="sel2r1")
    nc.vector.memset(sel2r1[:], 0.0)
    nc.vector.memset(sel2r1[0:1, 64:128], 1.0)
    nc.sync.dma_start(out=sel2[1:2, :], in_=sel2r1[:])

    def prow(i):   # partition block for sp/vt/ein chain
        return 64 * (i % 2)

    def pcol(i):   # col block index for sp
        return i // 2

    def qrow(i):   # partition block for k/q'
        return 32 * (i % 4)

    def qcol(i):
        return i // 4

    with tc.tile_pool(name="sp", bufs=2) as spp, \
         tc.tile_pool(name="spx", bufs=2) as spxp, \
         tc.tile_pool(name="kqt", bufs=3) as kqtp, \
         tc.tile_pool(name="ksb", bufs=2) as ksbp, \
         tc.tile_pool(name="vt", bufs=2) as vtp, \
         tc.tile_pool(name="qs", bufs=2) as qsp, \
         tc.tile_pool(name="qpad", bufs=2) as qpp, \
         tc.tile_pool(name="att", bufs=3) as attp, \
         tc.tile_pool(name="spbf", bufs=2) as spbfp, \
         tc.tile_pool(name="aps", bufs=1, space="PSUM") as aps:

        sp_all = None
        sp_bf = None
        for h in range(NH):
            if h == 0:
                sp_all = spp.tile([128, NI // 2, POS], f32, tag="sp", name="sp0")
                sp_bf = spbfp.tile([128, NI // 2, POS], bf16, tag="spbf",
                                   name="spbf0")
                for img in range(NI):
                    nc.gpsimd.dma_start(
                        out=sp_all[prow(img):prow(img) + 64, pcol(img), :],
                        in_=x2wm[0][0:64, img, :])
                for j in range(4):
                    for n2 in range(2):
                        nc.vector.tensor_copy(
                            sp_bf[:, j, 392 * n2:392 * (n2 + 1)],
                            sp_all[:, j, 392 * n2:392 * (n2 + 1)])
            spn = spn_bf = None
            if h < NH - 1:
                spn = spp.tile([128, NI // 2, POS], f32, tag="sp", name=f"sp{h + 1}")
                spn_bf = spbfp.tile([128, NI // 2, POS], bf16, tag="spbf",
                                    name=f"spbf{h + 1}")
                spx = spxp.tile([128, NI // 2, POS], bf16, tag="spx", name=f"spx{h}")
                c_next, half_next = (h + 1) // 2, (h + 1) % 2
                for img in range(NI):
                    nc.gpsimd.dma_start(
                        out=spx[prow(img):prow(img) + 64, pcol(img), :],
                        in_=x2wm[c_next][64 * half_next:64 * half_next + 64,
                                         img, :])

            k_pk = ksbp.tile([128, 2, POS], bf16, tag="k", name=f"k{h}")
            vt_pk = vtp.tile([128, (NI // 2) * NW * D], bf16, tag="vt", name=f"vt{h}")
            qstack = qsp.tile([128, POS], bf16, tag="qstack", name=f"qstack{h}")
            qp_pk = qsp.tile([128, 2, POS], bf16, tag="qp", name=f"qp{h}")

            # ---- B: kq + v projections, per image pair ----
            for j in range(4):
                kqt = kqtp.tile([128, POS], bf16, tag="kqt", name=f"kqt{h}_{j}")
                for n2 in range(2):
                    pkq = aps.tile([128, 512], f32, tag="bank", bufs=8,
                                   name=f"pkq{h}_{j}_{n2}")[:, 0:392]
                    pvt = aps.tile([128, 512], f32, tag="bank", bufs=8,
                                   name=f"pvt{h}_{j}_{n2}")
                    for i01 in range(2):
                        p_ = 64 * i01
                        nc.tensor.matmul(pkq[p_:p_ + 2 * KD, :],
                                         wkq_sb[h][p_:p_ + 64, :],
                                         sp_bf[p_:p_ + 64, j,
                                               392 * n2:392 * (n2 + 1)],
                                         start=True, stop=True,
                                         tile_position=(p_, p_))
                        for w in range(8):
                            wg = 8 * n2 + w
                            nc.tensor.matmul(pvt[p_:p_ + WN, 64 * w:64 * (w + 1)],
                                             sp_bf[p_:p_ + 64, j,
                                                   WN * wg:WN * (wg + 1)],
                                             wv_sb[h][p_:p_ + 64, :],
                                             start=True, stop=True,
                                             tile_position=(p_, p_))
                    nc.scalar.activation(kqt[:, 392 * n2:392 * (n2 + 1)], pkq[:],
                                         AF.Identity, bias=bkq_sb[h][:, 0:1])
                    nc.scalar.activation(
                        vt_pk[:, 1024 * j + 512 * n2:1024 * j + 512 * (n2 + 1)],
                        pvt[:], AF.Copy)
                for i01 in range(2):
                    img = 2 * j + i01
                    q_ = qrow(img)
                    nc.sync.dma_start(
                        out=k_pk[q_:q_ + KD, qcol(img), :],
                        in_=kqt[64 * i01:64 * i01 + KD, :])
                    nc.sync.dma_start(
                        out=qstack[KD * img:KD * (img + 1), :],
                        in_=kqt[64 * i01 + KD:64 * i01 + 2 * KD, :])

            # ---- C: depthwise conv on stacked q (two guttered half-grids) ----
            kk = KS[h]
            p = kk // 2
            CW = 7 + p                   # cell stride
            HH = 2 * CW + p              # half-grid rows
            SW = 4 * CW + p              # grid cols
            SP = SW + (SW % 2)           # pad col stride to even (bf16 align)
            Lh, Lw = HH - 2 * p, SW - 2 * p
            qsv = qstack[:].rearrange("q (n h w) -> q n h w", n=NW, h=7)
            qflat = qsp.tile([128, NW, 49], bf16, tag="qflat", name=f"qflat{h}")
            for n2 in range(2):
                G = qpp.tile([128, 23, 44], bf16, tag="qpad",
                             name=f"qpad{h}_{n2}", padded_shape=None)
                nc.vector.memset(G[:, 0:HH, 0:SP], 0.0)
                for a2 in range(2):
                    nc.vector.tensor_copy(
                        cell_ap(G, p + CW * a2, p, CW),
                        qsv[:, 8 * n2 + 4 * a2:8 * n2 + 4 * a2 + 4])
                GA = qpp.tile([128, 23, 44], bf16, tag="qacc",
                              name=f"qacc{h}_{n2}")
                nc.vector.memset(GA[:, p:p + Lh, 0:44], 0.0)
                for t, (dy, dx) in enumerate(_dw_taps(kk)):
                    srcap = G[:, dy:dy + Lh, dx:dx + Lw]
                    dstap = GA[:, p:p + Lh, p:p + Lw]
                    nc.vector.scalar_tensor_tensor(dstap, srcap,
                                                   dq_w[h][:, t:t + 1],
                                                   dstap, AO.mult, AO.add)
                for a2 in range(2):
                    nc.vector.tensor_copy(
                        qflat[:, 8 * n2 + 4 * a2:8 * n2 + 4 * a2 + 4]
                        .rearrange("q n (h w) -> q n h w", h=7),
                        cell_ap(GA, p + CW * a2, p, CW))
                qfb = qsp.tile([128, 392], bf16, tag="qfb",
                               name=f"qfb{h}_{n2}")
                nc.scalar.activation(
                    qfb[:], qflat[:, 8 * n2:8 * n2 + 8, :]
                    .rearrange("q n s -> q (n s)"),
                    AF.Identity, bias=dq_b[h][:, 0:1])
                for img in range(NI):
                    nc.sync.dma_start(
                        out=qp_pk[qrow(img):qrow(img) + KD, qcol(img),
                                  392 * n2:392 * (n2 + 1)],
                        in_=qfb[KD * img:KD * (img + 1), :])

            # ---- D: attention per (pair, half) ----
            cy, hy = h // 2, h % 2
            for j in range(4):
                for n2 in range(2):
                    co = 392 * n2
                    pa = aps.tile([128, 512], f32, tag="bank", bufs=8,
                                  name=f"pa{h}_{j}_{n2}")[:, 0:392]
                    for i01 in range(2):
                        img = 2 * j + i01
                        p_, q_ = 64 * i01, qrow(img)
                        for w in range(8):
                            wg = 8 * n2 + w
                            nc.tensor.matmul(
                                pa[p_:p_ + WN, WN * w:WN * (w + 1)],
                                k_pk[q_:q_ + KD, qcol(img), WN * wg:WN * (wg + 1)],
                                qp_pk[q_:q_ + KD, qcol(img), WN * wg:WN * (wg + 1)],
                                start=True, stop=True, tile_position=(q_, p_))
                    ein = attp.tile([128, 392], bf16, tag="ein",
                                    name=f"ein{h}_{j}_{n2}")
                    nc.vector.scalar_tensor_tensor(ein[:], pa[:], 20.0,
                                                   ab_sb[h][:], AO.min, AO.add)
                    eexp = attp.tile([128, 392], bf16, tag="eexp",
                                     name=f"eexp{h}_{j}_{n2}")
                    nc.scalar.activation(eexp[:], ein[:], AF.Exp)
                    ps1 = aps.tile([128, 512], f32, tag="bank", bufs=8,
                                   name=f"ps1{h}_{j}_{n2}")[0:2, 0:392]
                    nc.tensor.matmul(ps1[:], ones2[:], eexp[:],
                                     start=True, stop=True)
                    rs = attp.tile([2, 392], f32, tag="rs", name=f"rs{h}_{j}_{n2}")
                    nc.vector.reciprocal_approx_fast(rs[:], ps1[:])
                    pbc = aps.tile([128, 512], f32, tag="bank", bufs=8,
                                   name=f"pbc{h}_{j}_{n2}")[:, 0:392]
                    nc.tensor.matmul(pbc[:], sel2[:], rs[:],
                                     start=True, stop=True)
                    bc = attp.tile([128, 392], bf16, tag="bc",
                                   name=f"bc{h}_{j}_{n2}")
                    nc.scalar.activation(bc[:], pbc[:], AF.Copy)
                    pav = aps.tile([128, 512], f32, tag="bank", bufs=8,
                                   name=f"pav{h}_{j}_{n2}")[:, 0:392]
                    for i01 in range(2):
                        p_ = 64 * i01
                        for w in range(8):
                            wg = 8 * n2 + w
                            nc.tensor.matmul(
                                pav[p_:p_ + D, WN * w:WN * (w + 1)],
                                vt_pk[p_:p_ + WN,
                                      1024 * j + 64 * wg:1024 * j + 64 * (wg + 1)],
                                eexp[p_:p_ + WN, WN * w:WN * (w + 1)],
                                start=True, stop=True, tile_position=(p_, p_))
                    spo = attp.tile([128, 392], bf16, tag="spo",
                                    name=f"spo{h}_{j}_{n2}")
                    nc.vector.tensor_tensor(spo[:], pav[:], bc[:], AO.mult)
                    for i01 in range(2):
                        img = 2 * j + i01
                        nc.gpsimd.dma_start(
                            out=y_dram[cy, 64 * hy:64 * hy + 64, img, co:co + 392],
                            in_=spo[64 * i01:64 * i01 + 64, :])
                    if h < NH - 1:
                        nc.vector.scalar_tensor_tensor(
                            spn[:, j, co:co + 392], spo[:],
                            bv_sb[h][:, 0:1], spx[:, j, co:co + 392],
                            AO.add, AO.add)
                        nc.vector.tensor_copy(
                            spn_bf[:, j, co:co + 392],
                            spn[:, j, co:co + 392])
            sp_all = spn
            sp_bf = spn_bf

    atw_cm.__exit__(None, None, None)

    # -------- P4+P5+P6: y hswish + proj + dw1 + MLP1 + out, pipelined ------
    with tc.tile_pool(name="pjw", bufs=1) as pjw, \
         tc.tile_pool(name="hyp", bufs=3) as hyp, \
         tc.tile_pool(name="x2r", bufs=2) as x2rp, \
         tc.tile_pool(name="x3p", bufs=2) as x3p, \
         tc.tile_pool(name="dwp1", bufs=2) as padp1, \
         tc.tile_pool(name="dwa1", bufs=2) as accp1, \
         tc.tile_pool(name="x4p", bufs=2) as x4p, \
         tc.tile_pool(name="m1h", bufs=2) as hp1, \
         tc.tile_pool(name="m1r", bufs=3) as rp1, \
         tc.tile_pool(name="m1z", bufs=3) as zp1, \
         tc.tile_pool(name="o5", bufs=3) as o5p, \
         tc.tile_pool(name="ppp", bufs=2, space="PSUM") as ppp, \
         tc.tile_pool(name="m1ps", bufs=4, space="PSUM") as psp1, \
         tc.tile_pool(name="m1po", bufs=2, space="PSUM") as pop1:
        pj_sb = []
        for k in range(4):
            w = pjw.tile([128, ED], bf16, tag=f"pj{k}")
            nc.sync.dma_start(out=w, in_=projT_d[128 * k:128 * (k + 1), :])
            pj_sb.append(w)
        pjb_sb, yb_sb, yb05_sb = [], [], []
        for m in range(4):
            b = pjw.tile([128, 1], f32, tag=f"pjb{m}")
            nc.sync.dma_start(out=b, in_=projb_d[128 * m:128 * (m + 1)].unsqueeze(1))
            pjb_sb.append(b)
            b = pjw.tile([128, 1], f32, tag=f"ybt{m}")
            nc.sync.dma_start(out=b, in_=yb_d[128 * m:128 * (m + 1)].unsqueeze(1))
            yb_sb.append(b)
            b05 = pjw.tile([128, 1], f32, tag=f"yb05{m}")
            nc.vector.tensor_scalar(b05[:], yb_sb[m][:], 1.0 / 6.0, 0.5,
                                    AO.mult, AO.add)
            yb05_sb.append(b05)
        acts2 = pjw.tile([128, 1], f32, tag="acts2")
        nc.vector.memset(acts2, 1.0 / 6.0)
        w1sb_, b1sb_ = [], []
        for c in range(4):
            w = pjw.tile([128, 9], f32, tag=f"dw1w{c}")
            nc.sync.dma_start(out=w, in_=dw1w_d[c])
            b = pjw.tile([128, 1], f32, tag=f"dw1b{c}")
            nc.sync.dma_start(out=b, in_=dw1b_d[c].unsqueeze(1))
            w1sb_.append(w)
            b1sb_.append(b)
        w1m = []
        for k in range(4):
            w = pjw.tile([128, 2 * ED], bf16, tag=f"m1w1_{k}")
            nc.sync.dma_start(out=w, in_=w1T1_d[128 * k:128 * (k + 1), :])
            w1m.append(w)
        w2m = []
        for k in range(8):
            w = pjw.tile([128, ED], bf16, tag=f"m1w2_{k}")
            nc.sync.dma_start(out=w, in_=w2T1_d[128 * k:128 * (k + 1), :])
            w2m.append(w)
        b1row1 = pjw.tile([1, 2 * ED], bf16, tag="b1row1")
        nc.sync.dma_start(out=b1row1, in_=b1f1_d.unsqueeze(0))
        ones392b = pjw.tile([1, 392], bf16, tag="ones392b")
        nc.vector.memset(ones392b, 1.0)
        acth1 = pjw.tile([128, 1], f32, tag="acth1")
        nc.vector.memset(acth1, 0.5)
        b2m = []
        for m in range(4):
            b = pjw.tile([128, 1], f32, tag=f"m1b2_{m}")
            nc.sync.dma_start(out=b, in_=b2f1_d[128 * m:128 * (m + 1)].unsqueeze(1))
            b2m.append(b)

        for j in range(4):
            x3s = {}
            for i01 in range(2):
                img = 2 * j + i01
                hys = []
                for c in range(4):
                    yt = hyp.tile([128, POS], bf16, tag="yt", name=f"yt{c}_{img}")
                    nc.sync.dma_start(out=yt, in_=y_dram[c, :, img, :])
                    z = hyp.tile([128, POS], bf16, tag="z", name=f"z{c}_{img}")
                    nc.scalar.activation(z[:], yt[:], AF.Identity,
                                         bias=yb_sb[c][:, 0:1])
                    hy = hyp.tile([128, POS], bf16, tag=f"hy{c}",
                                  name=f"hy{c}_{img}")
                    for n2 in range(2):
                        r = rp1.tile([128, 392], bf16, tag="pr")
                        nc.scalar.activation(r[:], yt[:, 392 * n2:392 * (n2 + 1)],
                                             AF.Relu, scale=acts2[:, 0:1],
                                             bias=yb05_sb[c][:, 0:1])
                        nc.vector.scalar_tensor_tensor(
                            hy[:, 392 * n2:392 * (n2 + 1)], r[:], 1.0,
                            z[:, 392 * n2:392 * (n2 + 1)], AO.min, AO.mult)
                    hys.append(hy)
                x2rb = [x2wm[c][:, img, :] for c in range(4)]
                for mo in range(4):
                    x3wm = x3p.tile([128, NW, 49], bf16, tag=f"x3{mo}",
                                    name=f"x3{mo}_{img}")
                    x3s[(mo, i01)] = x3wm
                    for n2 in range(2):
                        pp = ppp.tile([128, 392], f32, tag="pp")
                        for k in range(4):
                            nc.tensor.matmul(pp[:],
                                             pj_sb[k][:, 128 * mo:128 * (mo + 1)],
                                             hys[k][:, 392 * n2:392 * (n2 + 1)],
                                             start=(k == 0), stop=False)
                        nc.tensor.matmul(pp[:], ident_sb[:],
                                         x2rb[mo][:, 392 * n2:392 * (n2 + 1)],
                                         start=False, stop=True)
                        nc.scalar.activation(
                            x3wm[:].rearrange("p a w -> p (a w)")
                            [:, 392 * n2:392 * (n2 + 1)], pp[:],
                            AF.Identity, bias=pjb_sb[mo][:, 0:1])

            # dw1 on the pair: x3 wm -> DRAM -> spatial reload, clipped taps
            x4s = {}
            for c in range(4):
                for i01 in range(2):
                    im = 2 * j + i01
                    x3sp = padp1.tile([128, 28, 28], bf16, tag="x3sp",
                                      name=f"x3sp{c}_{im}")
                    x3wmv = x3s[(c, i01)][:].rearrange(
                        "p nw s -> p (nw s)").rearrange(
                        "p (b a h w) -> p b a h w", b=4, a=4, h=7)
                    for b in range(4):
                        nc.gpsimd.tensor_copy(
                            x3sp[:, :, 7 * b:7 * (b + 1)]
                            .rearrange("p (a h) w -> p a h w", a=4),
                            x3wmv[:, b])
                    acc = accp1.tile([128, 28, 28], f32, tag="acc1",
                                     name=f"acc1_{c}_{im}")
                    first = True
                    for t, (dy, dx) in _dw_taps3_sorted():
                        r0, r1 = max(0, dy - 1), 28 + min(0, dy - 1)
                        c0, c1 = max(0, dx - 1), 28 + min(0, dx - 1)
                        src = x3sp[:, r0:r1, c0:c1]
                        dst = acc[:, max(0, 1 - dy):max(0, 1 - dy) + (r1 - r0),
                                  max(0, 1 - dx):max(0, 1 - dx) + (c1 - c0)]
                        if first:
                            # acc = w_center*x3 + x3 (residual folded in)
                            nc.vector.scalar_tensor_tensor(
                                dst, src, w1sb_[c][:, t:t + 1], x3sp[:],
                                AO.mult, AO.add)
                            first = False
                        else:
                            nc.vector.scalar_tensor_tensor(
                                dst, src, w1sb_[c][:, t:t + 1], dst,
                                AO.mult, AO.add)
                    x4 = x4p.tile([128, 28, 28], bf16, tag=f"x4_{c}",
                                  name=f"x4_{c}_{im}")
                    nc.vector.scalar_tensor_tensor(
                        x4[:], acc[:], b1sb_[c][:, 0:1], acc[:],
                        AO.add, AO.bypass)
                    x4s[(c, i01)] = x4

            for i01 in range(2):
                img = 2 * j + i01
                x4f = [x4s[(c, i01)][:].rearrange("p h w -> p (h w)")
                       for c in range(4)]
                hs = [hp1.tile([128, POS], bf16, tag=f"g{m}", name=f"g{m}_{img}")
                      for m in range(8)]
                for m in range(8):
                    for n2 in range(2):
                        ph = psp1.tile([128, 392], f32, tag="ph1")
                        for k in range(4):
                            nc.tensor.matmul(
                                ph[:], w1m[k][:, 128 * m:128 * (m + 1)],
                                x4f[k][:, 392 * n2:392 * (n2 + 1)],
                                start=(k == 0), stop=False)
                        nc.tensor.matmul(
                            ph[:], b1row1[:, 128 * m:128 * (m + 1)],
                            ones392b[:], start=False, stop=True)
                        r = rp1.tile([128, 392], bf16, tag="r1")
                        nc.scalar.activation(r[:], ph[:], AF.Relu,
                                             scale=acts2[:, 0:1],
                                             bias=acth1[:, 0:1])
                        nc.vector.scalar_tensor_tensor(
                            hs[m][:, 392 * n2:392 * (n2 + 1)], r[:], 1.0,
                            ph[:], AO.min, AO.mult)
                for mo in range(4):
                    for n2 in range(2):
                        po = pop1.tile([128, 392], f32, tag="po1")
                        for k in range(8):
                            nc.tensor.matmul(
                                po[:], w2m[k][:, 128 * mo:128 * (mo + 1)],
                                hs[k][:, 392 * n2:392 * (n2 + 1)],
                                start=(k == 0), stop=(k == 7))
                        x5 = o5p.tile([128, 392], f32, tag="x5",
                                      name=f"x5_{mo}_{img}_{n2}")
                        nc.vector.scalar_tensor_tensor(
                            x5[:], po[:], b2m[mo][:, 0:1],
                            x4f[mo][:, 392 * n2:392 * (n2 + 1)],
                            AO.add, AO.add)
                        nc.sync.dma_start(
                            out=out_d[img, 128 * mo:128 * (mo + 1),
                                      392 * n2:392 * (n2 + 1)],
                            in_=x5[:])

    misc_cm.__exit__(None, None, None)
    x2wm_cm.__exit__(None, None, None)
    dram_cm.__exit__(None, None, None)


# ---------------------------------------------------------------------------
# host-side input preprocessing
# ---------------------------------------------------------------------------

def prep_weights(inp):
    def taps(w):  # [C,1,k,k] -> [C, k*k]
        return w.reshape(w.shape[0], -1).astype(np.float32)

    m = {}
    m["dw0w"] = taps(inp["dw0_w"]).reshape(4, 128, 9)
    m["dw0b"] = inp["dw0_b"].reshape(4, 128).astype(np.float32)
    m["w1T0"] = np.ascontiguousarray(inp["ffn0_w1"].T).astype(ml_dtypes.bfloat16)
    m["b1f0"] = inp["ffn0_b1"].astype(ml_dtypes.bfloat16)
    m["w2T0"] = np.ascontiguousarray(inp["ffn0_w2"].T).astype(ml_dtypes.bfloat16)
    m["b2f0"] = inp["ffn0_b2"].astype(np.float32)

    qkv_w, qkv_b = inp["qkv_w"], inp["qkv_b"]
    # reorder rows: k(16:32) first, then q(0:16); v separate
    wkqT = np.empty((NH, D, 2 * KD), np.float32)
    bkq = np.empty((NH, 2 * KD), np.float32)
    wvT = np.empty((NH, D, D), np.float32)
    bv = np.empty((NH, D), np.float32)
    for h in range(NH):
        W = qkv_w[h]  # [96, 64]
        wkqT[h, :, 0:KD] = W[KD:2 * KD].T
        wkqT[h, :, KD:2 * KD] = W[0:KD].T
        bkq[h, 0:KD] = qkv_b[h, KD:2 * KD]
        bkq[h, KD:2 * KD] = qkv_b[h, 0:KD]
        wvT[h] = W[2 * KD:].T
        bv[h] = qkv_b[h, 2 * KD:]
    m["wkqT"] = wkqT.astype(ml_dtypes.bfloat16)
    m["bkq"] = bkq
    m["wvT"] = wvT.astype(ml_dtypes.bfloat16)
    m["bv"] = bv

    dwq_ws = [inp["dwq_w7"], inp["dwq_w5"]] + [inp["dwq_w3"][i] for i in range(6)]
    dwq_bs = [inp["dwq_b7"], inp["dwq_b5"]] + [inp["dwq_b3"][i] for i in range(6)]
    dwqw = np.zeros((NH, 128, 49), np.float32)
    dwqb = np.zeros((NH, 128), np.float32)
    for h in range(NH):
        t = taps(dwq_ws[h]) * SCALE          # [16, k*k]
        nt = t.shape[1]
        for i in range(NI):
            dwqw[h, KD * i:KD * (i + 1), :nt] = t
            dwqb[h, KD * i:KD * (i + 1)] = dwq_bs[h] * SCALE
    m["dwqw"] = dwqw
    m["dwqb"] = dwqb

    ab = inp["attn_bias"][:, BIAS_IDX]       # [NH, 49, 49]
    m["ab"] = np.tile(ab, (1, 1, 8)).astype(np.float32)  # [NH, 49, 392]

    m["projT"] = np.ascontiguousarray(inp["proj_w"].T).astype(ml_dtypes.bfloat16)
    m["projb"] = inp["proj_b"].astype(np.float32)
    m["yb"] = bv.reshape(ED).astype(np.float32)

    m["dw1w"] = taps(inp["dw1_w"]).reshape(4, 128, 9)
    m["dw1b"] = inp["dw1_b"].reshape(4, 128).astype(np.float32)
    m["w1T1"] = np.ascontiguousarray(inp["ffn1_w1"].T).astype(ml_dtypes.bfloat16)
    m["b1f1"] = inp["ffn1_b1"].astype(ml_dtypes.bfloat16)
    m["w2T1"] = np.ascontiguousarray(inp["ffn1_w2"].T).astype(ml_dtypes.bfloat16)
    m["b2f1"] = inp["ffn1_b2"].astype(np.float32)
    m["ident"] = np.eye(128, dtype=np.float32).astype(ml_dtypes.bfloat16)
    return m


@functools.lru_cache(maxsize=1)
def _cached_program():
    return build_program()


def _run(inputs, trace=False, **kw):
    nc = _cached_program()
    wm = prep_weights(inputs)
    x = np.asarray(inputs["x"], dtype=np.float32).reshape(64, ED, POS)
    in_maps = []
    for core in range(NCORES):
        im = dict(wm)
        im["x"] = np.ascontiguousarray(x[NI * core:NI * (core + 1)])
        in_maps.append(im)
    res = bass_utils.run_bass_kernel_spmd(nc, in_maps, list(range(NCORES)),
                                          trace=trace, **kw)
    out = np.concatenate([r["out"] for r in res.results], axis=0)
    return out.reshape(64, ED, RES, RES).astype(np.float32), res


def kernel(**inputs):
    out, _ = _run(inputs)
    return out



# revision 40
# speedup vs baseline: 1.2694x; 1.2694x over previous
"""Trainium2 Bass kernel for nn_BasicBlock (EfficientViT-style block).

Data-parallel over 8 NeuronCores: batch 64 -> 8 images/core.
Per-core program: dw0 -> MLP0 -> cascaded window attention -> proj -> dw1 -> MLP1.
"""
import itertools
import functools
import numpy as np
import ml_dtypes

import concourse.bass as bass
import concourse.mybir as mybir
import concourse.tile as tile
from concourse import bacc
from concourse import bass_utils

f32 = mybir.dt.float32
f32r = mybir.dt.float32r
bf16 = mybir.dt.bfloat16
AO = mybir.AluOpType
AF = mybir.ActivationFunctionType

ED, KD, NH, AR = 512, 16, 8, 4
D = AR * KD            # 64
DH = D * NH            # 512
RES, WS = 28, 7
SCALE = KD ** -0.5
KS = [7, 5, 3, 3, 3, 3, 3, 3]
NI = 8                 # images per core
NCORES = 8
POS = RES * RES        # 784
NW = 16                # windows per image
WN = WS * WS           # 49


def _bias_idx(ws):
    pts = list(itertools.product(range(ws), range(ws)))
    offs, idxs = {}, []
    for p1 in pts:
        for p2 in pts:
            o = (abs(p1[0] - p2[0]), abs(p1[1] - p2[1]))
            if o not in offs:
                offs[o] = len(offs)
            idxs.append(offs[o])
    return np.array(idxs, dtype=np.int32).reshape(ws * ws, ws * ws), len(offs)


BIAS_IDX, N_OFFS = _bias_idx(WS)


# ---------------------------------------------------------------------------
# program builder
# ---------------------------------------------------------------------------

def _dw_taps(k):
    return [(dy, dx) for dy in range(k) for dx in range(k)]


def _dw_taps3_sorted():
    """3x3 taps with the full-coverage center tap (1,1) first."""
    return sorted(enumerate(_dw_taps(3)),
                  key=lambda e: (e[1][0] != 1, e[1][1] != 1))


def cell_ap(tile_ap, r0, c0, cw, nb=4):
    """[nb, 7, 7] strided view of `nb` conv cells at rows r0.., cols c0+cw*b."""
    base = tile_ap[:, r0:r0 + 1, c0:c0 + 1]
    part = base.ap[0]
    return bass.AP(base.tensor, base.offset,
                   [[part[0], part[1]], [cw, nb],
                    [tile_ap.shape[2], 7], [1, 7]])


def build_program():
    nc = bacc.Bacc("TRN2", target_bir_lowering=False, debug=False,
                   enable_asserts=False, num_devices=NCORES)

    dt_in = {}

    def din(name, shape, dt=f32):
        t = nc.dram_tensor(name, list(shape), dt, kind="ExternalInput")
        dt_in[name] = t
        return t.ap()

    x_d = din("x", [NI, ED, POS])
    dw0w_d = din("dw0w", [4, 128, 9])
    dw0b_d = din("dw0b", [4, 128])
    w1T0_d = din("w1T0", [ED, 2 * ED], bf16)
    b1f0_d = din("b1f0", [2 * ED], bf16)
    w2T0_d = din("w2T0", [2 * ED, ED], bf16)
    b2f0_d = din("b2f0", [ED])
    wkqT_d = din("wkqT", [NH, D, 2 * KD], bf16)
    bkq_d = din("bkq", [NH, 2 * KD])
    wvT_d = din("wvT", [NH, D, D], bf16)
    bv_d = din("bv", [NH, D])
    dwqw_d = din("dwqw", [NH, 128, 49])
    dwqb_d = din("dwqb", [NH, 128])
    ab_d = din("ab", [NH, WN, 8 * WN])
    projT_d = din("projT", [DH, ED], bf16)
    projb_d = din("projb", [ED])
    yb_d = din("yb", [ED])
    dw1w_d = din("dw1w", [4, 128, 9])
    dw1b_d = din("dw1b", [4, 128])
    w1T1_d = din("w1T1", [ED, 2 * ED], bf16)
    b1f1_d = din("b1f1", [2 * ED], bf16)
    w2T1_d = din("w2T1", [2 * ED, ED], bf16)
    b2f1_d = din("b2f1", [ED])
    ident_d = din("ident", [128, 128], bf16)

    out_d = nc.dram_tensor("out", [NI, ED, POS], f32, kind="ExternalOutput").ap()

    with tile.TileContext(nc) as tc:
        _body(tc, nc, x_d, dw0w_d, dw0b_d, w1T0_d, b1f0_d, w2T0_d, b2f0_d,
              wkqT_d, bkq_d, wvT_d, bv_d, dwqw_d, dwqb_d, ab_d,
              projT_d, projb_d, yb_d, dw1w_d, dw1b_d,
              w1T1_d, b1f1_d, w2T1_d, b2f1_d, out_d, ident_d)

    nc.compile()
    return nc


def _dwconv_block(tc, nc, pads, accs, src_getter, wsb, bsb, dst_writer,
                  pad_dt=bf16):
    for c in range(4):
        for img in range(NI):
            pad = pads.tile([128, 30, 30], pad_dt, tag="dwpad",
                            name=f"pad_{c}_{img}")
            nc.gpsimd.memset(pad, 0.0)
            src_getter(c, img, pad)
            acc = accs.tile([128, 28, 28], bf16, tag="dwacc",
                            name=f"acc_{c}_{img}")
            first = True
            for t, (dy, dx) in enumerate(_dw_taps(3)):
                srcap = pad[:, dy:dy + 28, dx:dx + 28]
                if first:
                    nc.vector.tensor_scalar(acc[:], srcap, wsb[c][:, t:t + 1],
                                            bsb[c][:, 0:1], AO.mult, AO.add)
                    first = False
                else:
                    nc.vector.scalar_tensor_tensor(acc[:], srcap, wsb[c][:, t:t + 1],
                                                   acc[:], AO.mult, AO.add)
            dst_writer(c, img, acc, pad)


def _body(tc, nc, x_d, dw0w_d, dw0b_d, w1T0_d, b1f0_d, w2T0_d, b2f0_d,
          wkqT_d, bkq_d, wvT_d, bv_d, dwqw_d, dwqb_d, ab_d,
          projT_d, projb_d, yb_d, dw1w_d, dw1b_d,
          w1T1_d, b1f1_d, w2T1_d, b2f1_d, out_d, ident_d):
    ctx_pools = []

    # DRAM intermediate: x2 in window-major layout [4, 128, NI, 784]
    dram_cm = tc.tile_pool(name="dram", bufs=1, space="DRAM")
    dram = dram_cm.__enter__()
    # Window index convention: window W = 4*b + a for spatial cell (a, b).
    # x2 lives in SBUF as bf16 window-major for the whole kernel.
    x2wm_cm = tc.tile_pool(name="x2wm", bufs=1)
    x2wm_p = x2wm_cm.__enter__()
    x2wm = [x2wm_p.tile([128, NI, POS], bf16, tag=f"x2wm{c}",
                        name=f"x2wm{c}") for c in range(4)]

    misc_cm = tc.tile_pool(name="misc", bufs=1)
    misc = misc_cm.__enter__()
    ident_sb = misc.tile([128, 128], bf16, tag="ident")
    nc.sync.dma_start(out=ident_sb, in_=ident_d)

    # ---------------- persistent pools -------------------------------------
    xp_cm = tc.tile_pool(name="xp", bufs=1)
    xp = xp_cm.__enter__()
    x_sb = []
    for c in range(4):
        t = xp.tile([128, NI, 28, 28], f32, tag=f"x{c}")
        x_sb.append(t)
        for img in range(NI):
            nc.sync.dma_start(out=t[:, img], in_=x_d[img, 128 * c:128 * (c + 1), :]
                              .rearrange("p (h w) -> p h w", h=28))

    # -------- P1+P2: dw0 + MLP0 + window-major x2 store, pipelined ---------
    with tc.tile_pool(name="dwk0", bufs=1) as dwk, \
         tc.tile_pool(name="m0w", bufs=1) as wp, \
         tc.tile_pool(name="dwp0", bufs=2) as padp, \
         tc.tile_pool(name="dwa0", bufs=2) as accp, \
         tc.tile_pool(name="x1b", bufs=2) as x1bp, \
         tc.tile_pool(name="m0h", bufs=1) as hp, \
         tc.tile_pool(name="m0r", bufs=3) as rp, \
         tc.tile_pool(name="m0z", bufs=3) as zp, \
         tc.tile_pool(name="wms", bufs=2) as wmstp, \
         tc.tile_pool(name="m0ps", bufs=4, space="PSUM") as psp, \
         tc.tile_pool(name="m0po", bufs=2, space="PSUM") as pop:
        w0sb, b0sb = [], []
        for c in range(4):
            w = dwk.tile([128, 9], f32, tag=f"dw0w{c}")
            nc.sync.dma_start(out=w, in_=dw0w_d[c])
            b = dwk.tile([128, 1], f32, tag=f"dw0b{c}")
            nc.sync.dma_start(out=b, in_=dw0b_d[c].unsqueeze(1))
            w0sb.append(w)
            b0sb.append(b)
        w1sb = []
        for k in range(4):
            w = wp.tile([128, 2 * ED], bf16, tag=f"w1_{k}")
            nc.sync.dma_start(out=w, in_=w1T0_d[128 * k:128 * (k + 1), :])
            w1sb.append(w)
        w2sb = []
        for k in range(8):
            w = wp.tile([128, ED], bf16, tag=f"w2_{k}")
            nc.sync.dma_start(out=w, in_=w2T0_d[128 * k:128 * (k + 1), :])
            w2sb.append(w)
        b1row = wp.tile([1, 2 * ED], bf16, tag="b1row")
        nc.sync.dma_start(out=b1row, in_=b1f0_d.unsqueeze(0))
        ones392 = wp.tile([1, 392], bf16, tag="ones392")
        nc.vector.memset(ones392, 1.0)
        acth = wp.tile([128, 1], f32, tag="acth")
        nc.vector.memset(acth, 0.5)
        acts = wp.tile([128, 1], f32, tag="acts")
        nc.vector.memset(acts, 1.0 / 6.0)
        b2sb = []
        for m in range(4):
            b = wp.tile([128, 1], f32, tag=f"b2_{m}")
            nc.sync.dma_start(out=b, in_=b2f0_d[128 * m:128 * (m + 1)].unsqueeze(1))
            b2sb.append(b)

        for j in range(4):
            # dw0: taps read x_sb f32 directly with edge clipping
            for c in range(4):
                for i01 in range(2):
                    im = 2 * j + i01
                    xim = x_sb[c][:, im]
                    acc = accp.tile([128, 28, 28], f32, tag="acc",
                                    name=f"acc{c}_{im}")
                    first = True
                    for t, (dy, dx) in _dw_taps3_sorted():
                        r0, r1 = max(0, dy - 1), 28 + min(0, dy - 1)
                        c0, c1 = max(0, dx - 1), 28 + min(0, dx - 1)
                        src_ = xim[:, r0:r1, c0:c1]
                        dst = acc[:, max(0, 1 - dy):max(0, 1 - dy) + (r1 - r0),
                                  max(0, 1 - dx):max(0, 1 - dx) + (c1 - c0)]
                        if first:
                            # acc = w_center*x + x  (residual folded in)
                            nc.vector.scalar_tensor_tensor(
                                dst, src_, w0sb[c][:, t:t + 1], xim,
                                AO.mult, AO.add)
                            first = False
                        else:
                            nc.vector.scalar_tensor_tensor(
                                dst, src_, w0sb[c][:, t:t + 1], dst,
                                AO.mult, AO.add)
                    # x1 = acc + b0  -> in place (x already folded into acc)
                    nc.vector.scalar_tensor_tensor(
                        xim, acc[:], b0sb[c][:, 0:1], acc[:], AO.add, AO.bypass)
            for i01 in range(2):
                img = 2 * j + i01
                x1b = []
                for c in range(4):
                    t = x1bp.tile([128, POS], bf16, tag=f"x1b{c}",
                                  name=f"x1b{c}_{img}")
                    nc.vector.tensor_copy(
                        t[:], x_sb[c][:, img].rearrange("p h w -> p (h w)"))
                    x1b.append(t)
                hs = [hp.tile([128, POS], bf16, tag=f"h{m}", name=f"h{m}_{img}")
                      for m in range(8)]
                for m in range(8):
                    for n2 in range(2):
                        ph = psp.tile([128, 392], f32, tag="ph")
                        for k in range(4):
                            nc.tensor.matmul(
                                ph[:], w1sb[k][:, 128 * m:128 * (m + 1)],
                                x1b[k][:, 392 * n2:392 * (n2 + 1)],
                                start=(k == 0), stop=False)
                        nc.tensor.matmul(
                            ph[:], b1row[:, 128 * m:128 * (m + 1)],
                            ones392[:], start=False, stop=True)
                        r = rp.tile([128, 392], bf16, tag="relu")
                        nc.scalar.activation(r[:], ph[:], AF.Relu,
                                             scale=acts[:, 0:1], bias=acth[:, 0:1])
                        nc.vector.scalar_tensor_tensor(
                            hs[m][:, 392 * n2:392 * (n2 + 1)], r[:], 1.0,
                            ph[:], AO.min, AO.mult)
                for mo in range(4):
                    x2v = x_sb[mo][:, img].rearrange("p h w -> p (h w)")
                    for n2 in range(2):
                        po = pop.tile([128, 392], f32, tag="po")
                        for k in range(8):
                            nc.tensor.matmul(
                                po[:], w2sb[k][:, 128 * mo:128 * (mo + 1)],
                                hs[k][:, 392 * n2:392 * (n2 + 1)],
                                start=(k == 0), stop=False)
                        nc.tensor.matmul(
                            po[:], ident_sb[:],
                            x1b[mo][:, 392 * n2:392 * (n2 + 1)],
                            start=False, stop=True)
                        nc.scalar.activation(
                            x2v[:, 392 * n2:392 * (n2 + 1)], po[:],
                            AF.Identity, bias=b2sb[mo][:, 0:1])
                    stb = wmstp.tile([128, POS], bf16, tag="stb",
                                     name=f"stb{mo}_{img}")
                    nc.vector.tensor_copy(stb[:], x2v)
                    sbv = stb[:].rearrange("p (a h b w) -> p a h b w",
                                           a=4, h=7, b=4)
                    for b in range(4):
                        nc.gpsimd.tensor_copy(
                            x2wm[mo][:, img, 196 * b:196 * (b + 1)]
                            .rearrange("p (a h w) -> p a h w", a=4, h=7),
                            sbv[:, :, :, b, :])

    xp_cm.__exit__(None, None, None)

    # ---------------- P3: cascaded attention -------------------------------
    # Packing: image i -> partition block 64*(i%2); col block i//2 (sp, vt, ein)
    #          k/q' : image i -> partition block 32*(i%4); col block i//4
    y_dram = dram.tile([4, 128, NI, POS], bf16, name="y_dram")

    atw_cm = tc.tile_pool(name="atw", bufs=1)
    atw = atw_cm.__enter__()
    wkq_sb, bkq_sb, wv_sb, bv_sb, dq_w, dq_b, ab_sb = [], [], [], [], [], [], []
    for h in range(NH):
        t = atw.tile([128, 2 * KD], bf16, tag=f"wkq{h}", name=f"wkq{h}")
        nc.sync.dma_start(out=t[0:64, :], in_=wkqT_d[h])
        nc.sync.dma_start(out=t[64:128, :], in_=wkqT_d[h])
        wkq_sb.append(t)
        t = atw.tile([128, 1], f32, tag=f"bkq{h}", name=f"bkq{h}")
        nc.sync.dma_start(out=t[0:32, :], in_=bkq_d[h].unsqueeze(1))
        nc.sync.dma_start(out=t[64:96, :], in_=bkq_d[h].unsqueeze(1))
        bkq_sb.append(t)
        t = atw.tile([128, D], bf16, tag=f"wv{h}", name=f"wv{h}")
        nc.sync.dma_start(out=t[0:64, :], in_=wvT_d[h])
        nc.sync.dma_start(out=t[64:128, :], in_=wvT_d[h])
        wv_sb.append(t)
        t = atw.tile([128, 1], f32, tag=f"bv{h}", name=f"bv{h}")
        nc.sync.dma_start(out=t[0:64, :], in_=bv_d[h].unsqueeze(1))
        nc.sync.dma_start(out=t[64:128, :], in_=bv_d[h].unsqueeze(1))
        bv_sb.append(t)
        t = atw.tile([128, 49], f32, tag=f"dqw{h}", name=f"dqw{h}")
        nc.sync.dma_start(out=t, in_=dwqw_d[h])
        dq_w.append(t)
        t = atw.tile([128, 1], f32, tag=f"dqb{h}", name=f"dqb{h}")
        nc.sync.dma_start(out=t, in_=dwqb_d[h].unsqueeze(1))
        dq_b.append(t)
        t = atw.tile([128, 392], f32, tag=f"ab{h}", name=f"ab{h}")
        nc.vector.memset(t[:], 0.0)
        nc.sync.dma_start(out=t[0:49, :], in_=ab_d[h])
        nc.sync.dma_start(out=t[64:113, :], in_=ab_d[h])
        ab_sb.append(t)
    # softmax helpers for pair-batched layout (imgs 2j/2j+1 at rows 0/64)
    ones2 = atw.tile([128, 2], bf16, tag="ones2", name="ones2")
    nc.vector.memset(ones2, 0.0)
    nc.vector.memset(ones2[0:49, 0:1], 1.0)
    nc.vector.memset(ones2[64:113, 1:2], 1.0)
    sel2 = atw.tile([2, 128], f32, tag="sel2", name="sel2")
    nc.vector.memset(sel2[0:1, :], 0.0)
    nc.vector.memset(sel2[0:1, 0:64], 1.0)
    sel2r1 = atw.tile([1, 128], f32, tag="sel2r1", name="sel2r1")
    nc.vector.memset(sel2r1[:], 0.0)
    nc.vector.memset(sel2r1[0:1, 64:128], 1.0)
    nc.sync.dma_start(out=sel2[1:2, :], in_=sel2r1[:])

    def prow(i):   # partition block for sp/vt/ein chain
        return 64 * (i % 2)

    def pcol(i):   # col block index for sp
        return i // 2

    def qrow(i):   # partition block for k/q'
        return 32 * (i % 4)

    def qcol(i):
        return i // 4

    with tc.tile_pool(name="sp", bufs=2) as spp, \
         tc.tile_pool(name="spx", bufs=2) as spxp, \
         tc.tile_pool(name="kqt", bufs=3) as kqtp, \
         tc.tile_pool(name="ksb", bufs=2) as ksbp, \
         tc.tile_pool(name="vt", bufs=2) as vtp, \
         tc.tile_pool(name="qs", bufs=2) as qsp, \
         tc.tile_pool(name="qpad", bufs=2) as qpp, \
         tc.tile_pool(name="att", bufs=3) as attp, \
         tc.tile_pool(name="spbf", bufs=2) as spbfp, \
         tc.tile_pool(name="aps", bufs=1, space="PSUM") as aps:

        sp_all = None
        sp_bf = None
        for h in range(NH):
            if h == 0:
                sp_all = spp.tile([128, NI // 2, POS], f32, tag="sp", name="sp0")
                sp_bf = spbfp.tile([128, NI // 2, POS], bf16, tag="spbf",
                                   name="spbf0")
                for img in range(NI):
                    nc.gpsimd.dma_start(
                        out=sp_all[prow(img):prow(img) + 64, pcol(img), :],
                        in_=x2wm[0][0:64, img, :])
                for j in range(4):
                    for n2 in range(2):
                        nc.vector.tensor_copy(
                            sp_bf[:, j, 392 * n2:392 * (n2 + 1)],
                            sp_all[:, j, 392 * n2:392 * (n2 + 1)])
            spn = spn_bf = None
            if h < NH - 1:
                spn = spp.tile([128, NI // 2, POS], f32, tag="sp", name=f"sp{h + 1}")
                spn_bf = spbfp.tile([128, NI // 2, POS], bf16, tag="spbf",
                                    name=f"spbf{h + 1}")
                spx = spxp.tile([128, NI // 2, POS], bf16, tag="spx", name=f"spx{h}")
                c_next, half_next = (h + 1) // 2, (h + 1) % 2
                for img in range(NI):
                    nc.gpsimd.dma_start(
                        out=spx[prow(img):prow(img) + 64, pcol(img), :],
                        in_=x2wm[c_next][64 * half_next:64 * half_next + 64,
                                         img, :])

            k_pk = ksbp.tile([128, 2, POS], bf16, tag="k", name=f"k{h}")
            vt_pk = vtp.tile([128, (NI // 2) * NW * D], bf16, tag="vt", name=f"vt{h}")
            qstack = qsp.tile([128, POS], bf16, tag="qstack", name=f"qstack{h}")
            qp_pk = qsp.tile([128, 2, POS], bf16, tag="qp", name=f"qp{h}")

            # ---- B: kq + v projections, per image pair ----
            for j in range(4):
                kqt = kqtp.tile([128, POS], bf16, tag="kqt", name=f"kqt{h}_{j}")
                for n2 in range(2):
                    pkq = aps.tile([128, 392], f32, tag="pkq", bufs=1,
                                   name=f"pkq{h}_{j}_{n2}")
                    pvt = aps.tile([128, 512], f32, tag="pvt", bufs=1,
                                   name=f"pvt{h}_{j}_{n2}")
                    for i01 in range(2):
                        p_ = 64 * i01
                        nc.tensor.matmul(pkq[p_:p_ + 2 * KD, :],
                                         wkq_sb[h][p_:p_ + 64, :],
                                         sp_bf[p_:p_ + 64, j,
                                               392 * n2:392 * (n2 + 1)],
                                         start=True, stop=True,
                                         tile_position=(p_, p_))
                        for w in range(8):
                            wg = 8 * n2 + w
                            nc.tensor.matmul(pvt[p_:p_ + WN, 64 * w:64 * (w + 1)],
                                             sp_bf[p_:p_ + 64, j,
                                                   WN * wg:WN * (wg + 1)],
                                             wv_sb[h][p_:p_ + 64, :],
                                             start=True, stop=True,
                                             tile_position=(p_, p_))
                    nc.scalar.activation(kqt[:, 392 * n2:392 * (n2 + 1)], pkq[:],
                                         AF.Identity, bias=bkq_sb[h][:, 0:1])
                    nc.scalar.activation(
                        vt_pk[:, 1024 * j + 512 * n2:1024 * j + 512 * (n2 + 1)],
                        pvt[:], AF.Copy)
                for i01 in range(2):
                    img = 2 * j + i01
                    q_ = qrow(img)
                    nc.sync.dma_start(
                        out=k_pk[q_:q_ + KD, qcol(img), :],
                        in_=kqt[64 * i01:64 * i01 + KD, :])
                    nc.sync.dma_start(
                        out=qstack[KD * img:KD * (img + 1), :],
                        in_=kqt[64 * i01 + KD:64 * i01 + 2 * KD, :])

            # ---- C: depthwise conv on stacked q (two guttered half-grids) ----
            kk = KS[h]
            p = kk // 2
            CW = 7 + p                   # cell stride
            HH = 2 * CW + p              # half-grid rows
            SW = 4 * CW + p              # grid cols
            SP = SW + (SW % 2)           # pad col stride to even (bf16 align)
            Lh, Lw = HH - 2 * p, SW - 2 * p
            qsv = qstack[:].rearrange("q (n h w) -> q n h w", n=NW, h=7)
            qflat = qsp.tile([128, NW, 49], bf16, tag="qflat", name=f"qflat{h}")
            for n2 in range(2):
                G = qpp.tile([128, 23, 44], bf16, tag="qpad",
                             name=f"qpad{h}_{n2}", padded_shape=None)
                nc.vector.memset(G[:, 0:HH, 0:SP], 0.0)
                for a2 in range(2):
                    nc.vector.tensor_copy(
                        cell_ap(G, p + CW * a2, p, CW),
                        qsv[:, 8 * n2 + 4 * a2:8 * n2 + 4 * a2 + 4])
                GA = qpp.tile([128, 23, 44], bf16, tag="qacc",
                              name=f"qacc{h}_{n2}")
                nc.vector.memset(GA[:, p:p + Lh, 0:44], 0.0)
                for t, (dy, dx) in enumerate(_dw_taps(kk)):
                    srcap = G[:, dy:dy + Lh, dx:dx + Lw]
                    dstap = GA[:, p:p + Lh, p:p + Lw]
                    nc.vector.scalar_tensor_tensor(dstap, srcap,
                                                   dq_w[h][:, t:t + 1],
                                                   dstap, AO.mult, AO.add)
                for a2 in range(2):
                    nc.vector.tensor_copy(
                        qflat[:, 8 * n2 + 4 * a2:8 * n2 + 4 * a2 + 4]
                        .rearrange("q n (h w) -> q n h w", h=7),
                        cell_ap(GA, p + CW * a2, p, CW))
                qfb = qsp.tile([128, 392], bf16, tag="qfb",
                               name=f"qfb{h}_{n2}")
                nc.scalar.activation(
                    qfb[:], qflat[:, 8 * n2:8 * n2 + 8, :]
                    .rearrange("q n s -> q (n s)"),
                    AF.Identity, bias=dq_b[h][:, 0:1])
                for img in range(NI):
                    nc.sync.dma_start(
                        out=qp_pk[qrow(img):qrow(img) + KD, qcol(img),
                                  392 * n2:392 * (n2 + 1)],
                        in_=qfb[KD * img:KD * (img + 1), :])

            # ---- D: attention per (pair, half) ----
            cy, hy = h // 2, h % 2
            for j in range(4):
                for n2 in range(2):
                    co = 392 * n2
                    pa = aps.tile([128, 392], f32, tag="pa", bufs=2,
                                  name=f"pa{h}_{j}_{n2}")
                    for i01 in range(2):
                        img = 2 * j + i01
                        p_, q_ = 64 * i01, qrow(img)
                        for w in range(8):
                            wg = 8 * n2 + w
                            nc.tensor.matmul(
                                pa[p_:p_ + WN, WN * w:WN * (w + 1)],
                                k_pk[q_:q_ + KD, qcol(img), WN * wg:WN * (wg + 1)],
                                qp_pk[q_:q_ + KD, qcol(img), WN * wg:WN * (wg + 1)],
                                start=True, stop=True, tile_position=(q_, p_))
                    ein = attp.tile([128, 392], bf16, tag="ein",
                                    name=f"ein{h}_{j}_{n2}")
                    nc.vector.scalar_tensor_tensor(ein[:], pa[:], 20.0,
                                                   ab_sb[h][:], AO.min, AO.add)
                    eexp = attp.tile([128, 392], bf16, tag="eexp",
                                     name=f"eexp{h}_{j}_{n2}")
                    nc.scalar.activation(eexp[:], ein[:], AF.Exp)
                    ps1 = aps.tile([2, 392], f32, tag="ps1", bufs=1,
                                   name=f"ps1{h}_{j}_{n2}")
                    nc.tensor.matmul(ps1[:], ones2[:], eexp[:],
                                     start=True, stop=True)
                    rs = attp.tile([2, 392], f32, tag="rs", name=f"rs{h}_{j}_{n2}")
                    nc.vector.reciprocal_approx_fast(rs[:], ps1[:])
                    pbc = aps.tile([128, 392], f32, tag="pbc", bufs=1,
                                   name=f"pbc{h}_{j}_{n2}")
                    nc.tensor.matmul(pbc[:], sel2[:], rs[:],
                                     start=True, stop=True)
                    bc = attp.tile([128, 392], bf16, tag="bc",
                                   name=f"bc{h}_{j}_{n2}")
                    nc.scalar.activation(bc[:], pbc[:], AF.Copy)
                    pav = aps.tile([128, 392], f32, tag="pav", bufs=2,
                                   name=f"pav{h}_{j}_{n2}")
                    for i01 in range(2):
                        p_ = 64 * i01
                        for w in range(8):
                            wg = 8 * n2 + w
                            nc.tensor.matmul(
                                pav[p_:p_ + D, WN * w:WN * (w + 1)],
                                vt_pk[p_:p_ + WN,
                                      1024 * j + 64 * wg:1024 * j + 64 * (wg + 1)],
                                eexp[p_:p_ + WN, WN * w:WN * (w + 1)],
                                start=True, stop=True, tile_position=(p_, p_))
                    spo = attp.tile([128, 392], bf16, tag="spo",
                                    name=f"spo{h}_{j}_{n2}")
                    nc.vector.tensor_tensor(spo[:], pav[:], bc[:], AO.mult)
                    for i01 in range(2):
                        img = 2 * j + i01
                        nc.gpsimd.dma_start(
                            out=y_dram[cy, 64 * hy:64 * hy + 64, img, co:co + 392],
                            in_=spo[64 * i01:64 * i01 + 64, :])
                    if h < NH - 1:
                        nc.vector.scalar_tensor_tensor(
                            spn[:, j, co:co + 392], spo[:],
                            bv_sb[h][:, 0:1], spx[:, j, co:co + 392],
                            AO.add, AO.add)
                        nc.vector.tensor_copy(
                            spn_bf[:, j, co:co + 392],
                            spn[:, j, co:co + 392])
            sp_all = spn
            sp_bf = spn_bf

    atw_cm.__exit__(None, None, None)

    # -------- P4+P5+P6: y hswish + proj + dw1 + MLP1 + out, pipelined ------
    with tc.tile_pool(name="pjw", bufs=1) as pjw, \
         tc.tile_pool(name="hyp", bufs=3) as hyp, \
         tc.tile_pool(name="x2r", bufs=2) as x2rp, \
         tc.tile_pool(name="x3p", bufs=2) as x3p, \
         tc.tile_pool(name="dwp1", bufs=2) as padp1, \
         tc.tile_pool(name="dwa1", bufs=2) as accp1, \
         tc.tile_pool(name="x4p", bufs=2) as x4p, \
         tc.tile_pool(name="m1h", bufs=2) as hp1, \
         tc.tile_pool(name="m1r", bufs=3) as rp1, \
         tc.tile_pool(name="m1z", bufs=3) as zp1, \
         tc.tile_pool(name="o5", bufs=3) as o5p, \
         tc.tile_pool(name="ppp", bufs=2, space="PSUM") as ppp, \
         tc.tile_pool(name="m1ps", bufs=4, space="PSUM") as psp1, \
         tc.tile_pool(name="m1po", bufs=2, space="PSUM") as pop1:
        pj_sb = []
        for k in range(4):
            w = pjw.tile([128, ED], bf16, tag=f"pj{k}")
            nc.sync.dma_start(out=w, in_=projT_d[128 * k:128 * (k + 1), :])
            pj_sb.append(w)
        pjb_sb, yb_sb, yb05_sb = [], [], []
        for m in range(4):
            b = pjw.tile([128, 1], f32, tag=f"pjb{m}")
            nc.sync.dma_start(out=b, in_=projb_d[128 * m:128 * (m + 1)].unsqueeze(1))
            pjb_sb.append(b)
            b = pjw.tile([128, 1], f32, tag=f"ybt{m}")
            nc.sync.dma_start(out=b, in_=yb_d[128 * m:128 * (m + 1)].unsqueeze(1))
            yb_sb.append(b)
            b05 = pjw.tile([128, 1], f32, tag=f"yb05{m}")
            nc.vector.tensor_scalar(b05[:], yb_sb[m][:], 1.0 / 6.0, 0.5,
                                    AO.mult, AO.add)
            yb05_sb.append(b05)
        acts2 = pjw.tile([128, 1], f32, tag="acts2")
        nc.vector.memset(acts2, 1.0 / 6.0)
        w1sb_, b1sb_ = [], []
        for c in range(4):
            w = pjw.tile([128, 9], f32, tag=f"dw1w{c}")
            nc.sync.dma_start(out=w, in_=dw1w_d[c])
            b = pjw.tile([128, 1], f32, tag=f"dw1b{c}")
            nc.sync.dma_start(out=b, in_=dw1b_d[c].unsqueeze(1))
            w1sb_.append(w)
            b1sb_.append(b)
        w1m = []
        for k in range(4):
            w = pjw.tile([128, 2 * ED], bf16, tag=f"m1w1_{k}")
            nc.sync.dma_start(out=w, in_=w1T1_d[128 * k:128 * (k + 1), :])
            w1m.append(w)
        w2m = []
        for k in range(8):
            w = pjw.tile([128, ED], bf16, tag=f"m1w2_{k}")
            nc.sync.dma_start(out=w, in_=w2T1_d[128 * k:128 * (k + 1), :])
            w2m.append(w)
        b1row1 = pjw.tile([1, 2 * ED], bf16, tag="b1row1")
        nc.sync.dma_start(out=b1row1, in_=b1f1_d.unsqueeze(0))
        ones392b = pjw.tile([1, 392], bf16, tag="ones392b")
        nc.vector.memset(ones392b, 1.0)
        acth1 = pjw.tile([128, 1], f32, tag="acth1")
        nc.vector.memset(acth1, 0.5)
        b2m = []
        for m in range(4):
            b = pjw.tile([128, 1], f32, tag=f"m1b2_{m}")
            nc.sync.dma_start(out=b, in_=b2f1_d[128 * m:128 * (m + 1)].unsqueeze(1))
            b2m.append(b)

        for j in range(4):
            x3s = {}
            for i01 in range(2):
                img = 2 * j + i01
                hys = []
                for c in range(4):
                    yt = hyp.tile([128, POS], bf16, tag="yt", name=f"yt{c}_{img}")
                    nc.sync.dma_start(out=yt, in_=y_dram[c, :, img, :])
                    z = hyp.tile([128, POS], bf16, tag="z", name=f"z{c}_{img}")
                    nc.scalar.activation(z[:], yt[:], AF.Identity,
                                         bias=yb_sb[c][:, 0:1])
                    hy = hyp.tile([128, POS], bf16, tag=f"hy{c}",
                                  name=f"hy{c}_{img}")
                    for n2 in range(2):
                        r = rp1.tile([128, 392], bf16, tag="pr")
                        nc.scalar.activation(r[:], yt[:, 392 * n2:392 * (n2 + 1)],
                                             AF.Relu, scale=acts2[:, 0:1],
                                             bias=yb05_sb[c][:, 0:1])
                        nc.vector.scalar_tensor_tensor(
                            hy[:, 392 * n2:392 * (n2 + 1)], r[:], 1.0,
                            z[:, 392 * n2:392 * (n2 + 1)], AO.min, AO.mult)
                    hys.append(hy)
                x2rb = [x2wm[c][:, img, :] for c in range(4)]
                for mo in range(4):
                    x3wm = x3p.tile([128, NW, 49], bf16, tag=f"x3{mo}",
                                    name=f"x3{mo}_{img}")
                    x3s[(mo, i01)] = x3wm
                    for n2 in range(2):
                        pp = ppp.tile([128, 392], f32, tag="pp")
                        for k in range(4):
                            nc.tensor.matmul(pp[:],
                                             pj_sb[k][:, 128 * mo:128 * (mo + 1)],
                                             hys[k][:, 392 * n2:392 * (n2 + 1)],
                                             start=(k == 0), stop=False)
                        nc.tensor.matmul(pp[:], ident_sb[:],
                                         x2rb[mo][:, 392 * n2:392 * (n2 + 1)],
                                         start=False, stop=True)
                        nc.scalar.activation(
                            x3wm[:].rearrange("p a w -> p (a w)")
                            [:, 392 * n2:392 * (n2 + 1)], pp[:],
                            AF.Identity, bias=pjb_sb[mo][:, 0:1])

            # dw1 on the pair: x3 wm -> DRAM -> spatial reload, clipped taps
            x4s = {}
            for c in range(4):
                for i01 in range(2):
                    im = 2 * j + i01
                    x3sp = padp1.tile([128, 28, 28], bf16, tag="x3sp",
                                      name=f"x3sp{c}_{im}")
                    x3wmv = x3s[(c, i01)][:].rearrange(
                        "p nw s -> p (nw s)").rearrange(
                        "p (b a h w) -> p b a h w", b=4, a=4, h=7)
                    for b in range(4):
                        nc.gpsimd.tensor_copy(
                            x3sp[:, :, 7 * b:7 * (b + 1)]
                            .rearrange("p (a h) w -> p a h w", a=4),
                            x3wmv[:, b])
                    acc = accp1.tile([128, 28, 28], f32, tag="acc1",
                                     name=f"acc1_{c}_{im}")
                    first = True
                    for t, (dy, dx) in _dw_taps3_sorted():
                        r0, r1 = max(0, dy - 1), 28 + min(0, dy - 1)
                        c0, c1 = max(0, dx - 1), 28 + min(0, dx - 1)
                        src = x3sp[:, r0:r1, c0:c1]
                        dst = acc[:, max(0, 1 - dy):max(0, 1 - dy) + (r1 - r0),
                                  max(0, 1 - dx):max(0, 1 - dx) + (c1 - c0)]
                        if first:
                            # acc = w_center*x3 + x3 (residual folded in)
                            nc.vector.scalar_tensor_tensor(
                                dst, src, w1sb_[c][:, t:t + 1], x3sp[:],
                                AO.mult, AO.add)
                            first = False
                        else:
                            nc.vector.scalar_tensor_tensor(
                                dst, src, w1sb_[c][:, t:t + 1], dst,
                                AO.mult, AO.add)
                    x4 = x4p.tile([128, 28, 28], bf16, tag=f"x4_{c}",
                                  name=f"x4_{c}_{im}")
                    nc.vector.scalar_tensor_tensor(
                        x4[:], acc[:], b1sb_[c][:, 0:1], acc[:],
                        AO.add, AO.bypass)
                    x4s[(c, i01)] = x4

            for i01 in range(2):
                img = 2 * j + i01
                x4f = [x4s[(c, i01)][:].rearrange("p h w -> p (h w)")
                       for c in range(4)]
                hs = [hp1.tile([128, POS], bf16, tag=f"g{m}", name=f"g{m}_{img}")
                      for m in range(8)]
                for m in range(8):
                    for n2 in range(2):
                        ph = psp1.tile([128, 392], f32, tag="ph1")
                        for k in range(4):
                            nc.tensor.matmul(
                                ph[:], w1m[k][:, 128 * m:128 * (m + 1)],
                                x4f[k][:, 392 * n2:392 * (n2 + 1)],
                                start=(k == 0), stop=False)
                        nc.tensor.matmul(
                            ph[:], b1row1[:, 128 * m:128 * (m + 1)],
                            ones392b[:], start=False, stop=True)
                        r = rp1.tile([128, 392], bf16, tag="r1")
                        nc.scalar.activation(r[:], ph[:], AF.Relu,
                                             scale=acts2[:, 0:1],
                                             bias=acth1[:, 0:1])
                        nc.vector.scalar_tensor_tensor(
                            hs[m][:, 392 * n2:392 * (n2 + 1)], r[:], 1.0,
                            ph[:], AO.min, AO.mult)
                for mo in range(4):
                    for n2 in range(2):
                        po = pop1.tile([128, 392], f32, tag="po1")
                        for k in range(8):
                            nc.tensor.matmul(
                                po[:], w2m[k][:, 128 * mo:128 * (mo + 1)],
                                hs[k][:, 392 * n2:392 * (n2 + 1)],
                                start=(k == 0), stop=(k == 7))
                        x5 = o5p.tile([128, 392], f32, tag="x5",
                                      name=f"x5_{mo}_{img}_{n2}")
                        nc.vector.scalar_tensor_tensor(
                            x5[:], po[:], b2m[mo][:, 0:1],
                            x4f[mo][:, 392 * n2:392 * (n2 + 1)],
                            AO.add, AO.add)
                        nc.sync.dma_start(
                            out=out_d[img, 128 * mo:128 * (mo + 1),
                                      392 * n2:392 * (n2 + 1)],
                            in_=x5[:])

    misc_cm.__exit__(None, None, None)
    x2wm_cm.__exit__(None, None, None)
    dram_cm.__exit__(None, None, None)


# ---------------------------------------------------------------------------
# host-side input preprocessing
# ---------------------------------------------------------------------------

def prep_weights(inp):
    def taps(w):  # [C,1,k,k] -> [C, k*k]
        return w.reshape(w.shape[0], -1).astype(np.float32)

    m = {}
    m["dw0w"] = taps(inp["dw0_w"]).reshape(4, 128, 9)
    m["dw0b"] = inp["dw0_b"].reshape(4, 128).astype(np.float32)
    m["w1T0"] = np.ascontiguousarray(inp["ffn0_w1"].T).astype(ml_dtypes.bfloat16)
    m["b1f0"] = inp["ffn0_b1"].astype(ml_dtypes.bfloat16)
    m["w2T0"] = np.ascontiguousarray(inp["ffn0_w2"].T).astype(ml_dtypes.bfloat16)
    m["b2f0"] = inp["ffn0_b2"].astype(np.float32)

    qkv_w, qkv_b = inp["qkv_w"], inp["qkv_b"]
    # reorder rows: k(16:32) first, then q(0:16); v separate
    wkqT = np.empty((NH, D, 2 * KD), np.float32)
    bkq = np.empty((NH, 2 * KD), np.float32)
    wvT = np.empty((NH, D, D), np.float32)
    bv = np.empty((NH, D), np.float32)
    for h in range(NH):
        W = qkv_w[h]  # [96, 64]
        wkqT[h, :, 0:KD] = W[KD:2 * KD].T
        wkqT[h, :, KD:2 * KD] = W[0:KD].T
        bkq[h, 0:KD] = qkv_b[h, KD:2 * KD]
        bkq[h, KD:2 * KD] = qkv_b[h, 0:KD]
        wvT[h] = W[2 * KD:].T
        bv[h] = qkv_b[h, 2 * KD:]
    m["wkqT"] = wkqT.astype(ml_dtypes.bfloat16)
    m["bkq"] = bkq
    m["wvT"] = wvT.astype(ml_dtypes.bfloat16)
    m["bv"] = bv

    dwq_ws = [inp["dwq_w7"], inp["dwq_w5"]] + [inp["dwq_w3"][i] for i in range(6)]
    dwq_bs = [inp["dwq_b7"], inp["dwq_b5"]] + [inp["dwq_b3"][i] for i in range(6)]
    dwqw = np.zeros((NH, 128, 49), np.float32)
    dwqb = np.zeros((NH, 128), np.float32)
    for h in range(NH):
        t = taps(dwq_ws[h]) * SCALE          # [16, k*k]
        nt = t.shape[1]
        for i in range(NI):
            dwqw[h, KD * i:KD * (i + 1), :nt] = t
            dwqb[h, KD * i:KD * (i + 1)] = dwq_bs[h] * SCALE
    m["dwqw"] = dwqw
    m["dwqb"] = dwqb

    ab = inp["attn_bias"][:, BIAS_IDX]       # [NH, 49, 49]
    m["ab"] = np.tile(ab, (1, 1, 8)).astype(np.float32)  # [NH, 49, 392]

    m["projT"] = np.ascontiguousarray(inp["proj_w"].T).astype(ml_dtypes.bfloat16)
    m["projb"] = inp["proj_b"].astype(np.float32)
    m["yb"] = bv.reshape(ED).astype(np.float32)

    m["dw1w"] = taps(inp["dw1_w"]).reshape(4, 128, 9)
    m["dw1b"] = inp["dw1_b"].reshape(4, 128).astype(np.float32)
    m["w1T1"] = np.ascontiguousarray(inp["ffn1_w1"].T).astype(ml_dtypes.bfloat16)
    m["b1f1"] = inp["ffn1_b1"].astype(ml_dtypes.bfloat16)
    m["w2T1"] = np.ascontiguousarray(inp["ffn1_w2"].T).astype(ml_dtypes.bfloat16)
    m["b2f1"] = inp["ffn1_b2"].astype(np.float32)
    m["ident"] = np.eye(128, dtype=np.float32).astype(ml_dtypes.bfloat16)
    return m


@functools.lru_cache(maxsize=1)
def _cached_program():
    return build_program()


def _run(inputs, trace=False, **kw):
    nc = _cached_program()
    wm = prep_weights(inputs)
    x = np.asarray(inputs["x"], dtype=np.float32).reshape(64, ED, POS)
    in_maps = []
    for core in range(NCORES):
        im = dict(wm)
        im["x"] = np.ascontiguousarray(x[NI * core:NI * (core + 1)])
        in_maps.append(im)
    res = bass_utils.run_bass_kernel_spmd(nc, in_maps, list(range(NCORES)),
                                          trace=trace, **kw)
    out = np.concatenate([r["out"] for r in res.results], axis=0)
    return out.reshape(64, ED, RES, RES).astype(np.float32), res


def kernel(**inputs):
    out, _ = _run(inputs)
    return out



# revision 41
# speedup vs baseline: 1.2728x; 1.0027x over previous
"""Trainium2 Bass kernel for nn_BasicBlock (EfficientViT-style block).

Data-parallel over 8 NeuronCores: batch 64 -> 8 images/core.
Per-core program: dw0 -> MLP0 -> cascaded window attention -> proj -> dw1 -> MLP1.
"""
import itertools
import functools
import numpy as np
import ml_dtypes

import concourse.bass as bass
import concourse.mybir as mybir
import concourse.tile as tile
from concourse import bacc
from concourse import bass_utils

f32 = mybir.dt.float32
f32r = mybir.dt.float32r
bf16 = mybir.dt.bfloat16
AO = mybir.AluOpType
AF = mybir.ActivationFunctionType

ED, KD, NH, AR = 512, 16, 8, 4
D = AR * KD            # 64
DH = D * NH            # 512
RES, WS = 28, 7
SCALE = KD ** -0.5
KS = [7, 5, 3, 3, 3, 3, 3, 3]
NI = 8                 # images per core
NCORES = 8
POS = RES * RES        # 784
NW = 16                # windows per image
WN = WS * WS           # 49


def _bias_idx(ws):
    pts = list(itertools.product(range(ws), range(ws)))
    offs, idxs = {}, []
    for p1 in pts:
        for p2 in pts:
            o = (abs(p1[0] - p2[0]), abs(p1[1] - p2[1]))
            if o not in offs:
                offs[o] = len(offs)
            idxs.append(offs[o])
    return np.array(idxs, dtype=np.int32).reshape(ws * ws, ws * ws), len(offs)


BIAS_IDX, N_OFFS = _bias_idx(WS)


# ---------------------------------------------------------------------------
# program builder
# ---------------------------------------------------------------------------

def _dw_taps(k):
    return [(dy, dx) for dy in range(k) for dx in range(k)]


def _dw_taps3_sorted():
    """3x3 taps with the full-coverage center tap (1,1) first."""
    return sorted(enumerate(_dw_taps(3)),
                  key=lambda e: (e[1][0] != 1, e[1][1] != 1))


def cell_ap(tile_ap, r0, c0, cw, nb=4):
    """[nb, 7, 7] strided view of `nb` conv cells at rows r0.., cols c0+cw*b."""
    base = tile_ap[:, r0:r0 + 1, c0:c0 + 1]
    part = base.ap[0]
    return bass.AP(base.tensor, base.offset,
                   [[part[0], part[1]], [cw, nb],
                    [tile_ap.shape[2], 7], [1, 7]])


def build_program():
    nc = bacc.Bacc("TRN2", target_bir_lowering=False, debug=False,
                   enable_asserts=False, num_devices=NCORES)

    dt_in = {}

    def din(name, shape, dt=f32):
        t = nc.dram_tensor(name, list(shape), dt, kind="ExternalInput")
        dt_in[name] = t
        return t.ap()

    x_d = din("x", [NI, ED, POS])
    dw0w_d = din("dw0w", [4, 128, 9])
    dw0b_d = din("dw0b", [4, 128])
    w1T0_d = din("w1T0", [ED, 2 * ED], bf16)
    b1f0_d = din("b1f0", [2 * ED], bf16)
    w2T0_d = din("w2T0", [2 * ED, ED], bf16)
    b2f0_d = din("b2f0", [ED])
    wkqT_d = din("wkqT", [NH, D, 2 * KD], bf16)
    bkq_d = din("bkq", [NH, 2 * KD])
    wvT_d = din("wvT", [NH, D, D], bf16)
    bv_d = din("bv", [NH, D])
    dwqw_d = din("dwqw", [NH, 128, 49])
    dwqb_d = din("dwqb", [NH, 128])
    ab_d = din("ab", [NH, WN, 8 * WN])
    projT_d = din("projT", [DH, ED], bf16)
    projb_d = din("projb", [ED])
    yb_d = din("yb", [ED])
    dw1w_d = din("dw1w", [4, 128, 9])
    dw1b_d = din("dw1b", [4, 128])
    w1T1_d = din("w1T1", [ED, 2 * ED], bf16)
    b1f1_d = din("b1f1", [2 * ED], bf16)
    w2T1_d = din("w2T1", [2 * ED, ED], bf16)
    b2f1_d = din("b2f1", [ED])
    ident_d = din("ident", [128, 128], bf16)

    out_d = nc.dram_tensor("out", [NI, ED, POS], f32, kind="ExternalOutput").ap()

    with tile.TileContext(nc) as tc:
        _body(tc, nc, x_d, dw0w_d, dw0b_d, w1T0_d, b1f0_d, w2T0_d, b2f0_d,
              wkqT_d, bkq_d, wvT_d, bv_d, dwqw_d, dwqb_d, ab_d,
              projT_d, projb_d, yb_d, dw1w_d, dw1b_d,
              w1T1_d, b1f1_d, w2T1_d, b2f1_d, out_d, ident_d)

    nc.compile()
    return nc


def _dwconv_block(tc, nc, pads, accs, src_getter, wsb, bsb, dst_writer,
                  pad_dt=bf16):
    for c in range(4):
        for img in range(NI):
            pad = pads.tile([128, 30, 30], pad_dt, tag="dwpad",
                            name=f"pad_{c}_{img}")
            nc.gpsimd.memset(pad, 0.0)
            src_getter(c, img, pad)
            acc = accs.tile([128, 28, 28], bf16, tag="dwacc",
                            name=f"acc_{c}_{img}")
            first = True
            for t, (dy, dx) in enumerate(_dw_taps(3)):
                srcap = pad[:, dy:dy + 28, dx:dx + 28]
                if first:
                    nc.vector.tensor_scalar(acc[:], srcap, wsb[c][:, t:t + 1],
                                            bsb[c][:, 0:1], AO.mult, AO.add)
                    first = False
                else:
                    nc.vector.scalar_tensor_tensor(acc[:], srcap, wsb[c][:, t:t + 1],
                                                   acc[:], AO.mult, AO.add)
            dst_writer(c, img, acc, pad)


def _body(tc, nc, x_d, dw0w_d, dw0b_d, w1T0_d, b1f0_d, w2T0_d, b2f0_d,
          wkqT_d, bkq_d, wvT_d, bv_d, dwqw_d, dwqb_d, ab_d,
          projT_d, projb_d, yb_d, dw1w_d, dw1b_d,
          w1T1_d, b1f1_d, w2T1_d, b2f1_d, out_d, ident_d):
    ctx_pools = []

    # DRAM intermediate: x2 in window-major layout [4, 128, NI, 784]
    dram_cm = tc.tile_pool(name="dram", bufs=1, space="DRAM")
    dram = dram_cm.__enter__()
    # Window index convention: window W = 4*b + a for spatial cell (a, b).
    # x2 lives in SBUF as bf16 window-major for the whole kernel.
    x2wm_cm = tc.tile_pool(name="x2wm", bufs=1)
    x2wm_p = x2wm_cm.__enter__()
    x2wm = [x2wm_p.tile([128, NI, POS], bf16, tag=f"x2wm{c}",
                        name=f"x2wm{c}") for c in range(4)]

    misc_cm = tc.tile_pool(name="misc", bufs=1)
    misc = misc_cm.__enter__()
    ident_sb = misc.tile([128, 128], bf16, tag="ident")
    nc.sync.dma_start(out=ident_sb, in_=ident_d)

    # ---------------- persistent pools -------------------------------------
    xp_cm = tc.tile_pool(name="xp", bufs=1)
    xp = xp_cm.__enter__()
    x_sb = []
    for c in range(4):
        t = xp.tile([128, NI, 28, 28], f32, tag=f"x{c}")
        x_sb.append(t)
        for img in range(NI):
            nc.sync.dma_start(out=t[:, img], in_=x_d[img, 128 * c:128 * (c + 1), :]
                              .rearrange("p (h w) -> p h w", h=28))

    # -------- P1+P2: dw0 + MLP0 + window-major x2 store, pipelined ---------
    with tc.tile_pool(name="dwk0", bufs=1) as dwk, \
         tc.tile_pool(name="m0w", bufs=1) as wp, \
         tc.tile_pool(name="dwp0", bufs=2) as padp, \
         tc.tile_pool(name="dwa0", bufs=3) as accp, \
         tc.tile_pool(name="x1b", bufs=2) as x1bp, \
         tc.tile_pool(name="m0h", bufs=1) as hp, \
         tc.tile_pool(name="m0r", bufs=3) as rp, \
         tc.tile_pool(name="m0z", bufs=3) as zp, \
         tc.tile_pool(name="wms", bufs=2) as wmstp, \
         tc.tile_pool(name="m0ps", bufs=4, space="PSUM") as psp, \
         tc.tile_pool(name="m0po", bufs=2, space="PSUM") as pop:
        w0sb, b0sb = [], []
        for c in range(4):
            w = dwk.tile([128, 9], f32, tag=f"dw0w{c}")
            nc.sync.dma_start(out=w, in_=dw0w_d[c])
            b = dwk.tile([128, 1], f32, tag=f"dw0b{c}")
            nc.sync.dma_start(out=b, in_=dw0b_d[c].unsqueeze(1))
            w0sb.append(w)
            b0sb.append(b)
        w1sb = []
        for k in range(4):
            w = wp.tile([128, 2 * ED], bf16, tag=f"w1_{k}")
            nc.sync.dma_start(out=w, in_=w1T0_d[128 * k:128 * (k + 1), :])
            w1sb.append(w)
        w2sb = []
        for k in range(8):
            w = wp.tile([128, ED], bf16, tag=f"w2_{k}")
            nc.sync.dma_start(out=w, in_=w2T0_d[128 * k:128 * (k + 1), :])
            w2sb.append(w)
        b1row = wp.tile([1, 2 * ED], bf16, tag="b1row")
        nc.sync.dma_start(out=b1row, in_=b1f0_d.unsqueeze(0))
        ones392 = wp.tile([1, 392], bf16, tag="ones392")
        nc.vector.memset(ones392, 1.0)
        acth = wp.tile([128, 1], f32, tag="acth")
        nc.vector.memset(acth, 0.5)
        acts = wp.tile([128, 1], f32, tag="acts")
        nc.vector.memset(acts, 1.0 / 6.0)
        b2sb = []
        for m in range(4):
            b = wp.tile([128, 1], f32, tag=f"b2_{m}")
            nc.sync.dma_start(out=b, in_=b2f0_d[128 * m:128 * (m + 1)].unsqueeze(1))
            b2sb.append(b)

        for j in range(4):
            # dw0: taps read x_sb f32 directly with edge clipping
            for c in range(4):
                for i01 in range(2):
                    im = 2 * j + i01
                    xim = x_sb[c][:, im]
                    acc = accp.tile([128, 28, 28], f32, tag="acc",
                                    name=f"acc{c}_{im}")
                    first = True
                    for t, (dy, dx) in _dw_taps3_sorted():
                        r0, r1 = max(0, dy - 1), 28 + min(0, dy - 1)
                        c0, c1 = max(0, dx - 1), 28 + min(0, dx - 1)
                        src_ = xim[:, r0:r1, c0:c1]
                        dst = acc[:, max(0, 1 - dy):max(0, 1 - dy) + (r1 - r0),
                                  max(0, 1 - dx):max(0, 1 - dx) + (c1 - c0)]
                        if first:
                            # acc = w_center*x + x  (residual folded in)
                            nc.vector.scalar_tensor_tensor(
                                dst, src_, w0sb[c][:, t:t + 1], xim,
                                AO.mult, AO.add)
                            first = False
                        else:
                            nc.vector.scalar_tensor_tensor(
                                dst, src_, w0sb[c][:, t:t + 1], dst,
                                AO.mult, AO.add)
                    # x1 = acc + b0  -> in place (x already folded into acc)
                    nc.vector.scalar_tensor_tensor(
                        xim, acc[:], b0sb[c][:, 0:1], acc[:], AO.add, AO.bypass)
            for i01 in range(2):
                img = 2 * j + i01
                x1b = []
                for c in range(4):
                    t = x1bp.tile([128, POS], bf16, tag=f"x1b{c}",
                                  name=f"x1b{c}_{img}")
                    nc.vector.tensor_copy(
                        t[:], x_sb[c][:, img].rearrange("p h w -> p (h w)"))
                    x1b.append(t)
                hs = [hp.tile([128, POS], bf16, tag=f"h{m}", name=f"h{m}_{img}")
                      for m in range(8)]
                for m in range(8):
                    for n2 in range(2):
                        ph = psp.tile([128, 392], f32, tag="ph")
                        for k in range(4):
                            nc.tensor.matmul(
                                ph[:], w1sb[k][:, 128 * m:128 * (m + 1)],
                                x1b[k][:, 392 * n2:392 * (n2 + 1)],
                                start=(k == 0), stop=False)
                        nc.tensor.matmul(
                            ph[:], b1row[:, 128 * m:128 * (m + 1)],
                            ones392[:], start=False, stop=True)
                        r = rp.tile([128, 392], bf16, tag="relu")
                        nc.scalar.activation(r[:], ph[:], AF.Relu,
                                             scale=acts[:, 0:1], bias=acth[:, 0:1])
                        nc.vector.scalar_tensor_tensor(
                            hs[m][:, 392 * n2:392 * (n2 + 1)], r[:], 1.0,
                            ph[:], AO.min, AO.mult)
                for mo in range(4):
                    x2v = x_sb[mo][:, img].rearrange("p h w -> p (h w)")
                    for n2 in range(2):
                        po = pop.tile([128, 392], f32, tag="po")
                        for k in range(8):
                            nc.tensor.matmul(
                                po[:], w2sb[k][:, 128 * mo:128 * (mo + 1)],
                                hs[k][:, 392 * n2:392 * (n2 + 1)],
                                start=(k == 0), stop=False)
                        nc.tensor.matmul(
                            po[:], ident_sb[:],
                            x1b[mo][:, 392 * n2:392 * (n2 + 1)],
                            start=False, stop=True)
                        nc.scalar.activation(
                            x2v[:, 392 * n2:392 * (n2 + 1)], po[:],
                            AF.Identity, bias=b2sb[mo][:, 0:1])
                    stb = wmstp.tile([128, POS], bf16, tag="stb",
                                     name=f"stb{mo}_{img}")
                    nc.vector.tensor_copy(stb[:], x2v)
                    sbv = stb[:].rearrange("p (a h b w) -> p a h b w",
                                           a=4, h=7, b=4)
                    for b in range(4):
                        nc.gpsimd.tensor_copy(
                            x2wm[mo][:, img, 196 * b:196 * (b + 1)]
                            .rearrange("p (a h w) -> p a h w", a=4, h=7),
                            sbv[:, :, :, b, :])

    xp_cm.__exit__(None, None, None)

    # ---------------- P3: cascaded attention -------------------------------
    # Packing: image i -> partition block 64*(i%2); col block i//2 (sp, vt, ein)
    #          k/q' : image i -> partition block 32*(i%4); col block i//4
    y_dram = dram.tile([4, 128, NI, POS], bf16, name="y_dram")

    atw_cm = tc.tile_pool(name="atw", bufs=1)
    atw = atw_cm.__enter__()
    wkq_sb, bkq_sb, wv_sb, bv_sb, dq_w, dq_b, ab_sb = [], [], [], [], [], [], []
    for h in range(NH):
        t = atw.tile([128, 2 * KD], bf16, tag=f"wkq{h}", name=f"wkq{h}")
        nc.sync.dma_start(out=t[0:64, :], in_=wkqT_d[h])
        nc.sync.dma_start(out=t[64:128, :], in_=wkqT_d[h])
        wkq_sb.append(t)
        t = atw.tile([128, 1], f32, tag=f"bkq{h}", name=f"bkq{h}")
        nc.sync.dma_start(out=t[0:32, :], in_=bkq_d[h].unsqueeze(1))
        nc.sync.dma_start(out=t[64:96, :], in_=bkq_d[h].unsqueeze(1))
        bkq_sb.append(t)
        t = atw.tile([128, D], bf16, tag=f"wv{h}", name=f"wv{h}")
        nc.sync.dma_start(out=t[0:64, :], in_=wvT_d[h])
        nc.sync.dma_start(out=t[64:128, :], in_=wvT_d[h])
        wv_sb.append(t)
        t = atw.tile([128, 1], f32, tag=f"bv{h}", name=f"bv{h}")
        nc.sync.dma_start(out=t[0:64, :], in_=bv_d[h].unsqueeze(1))
        nc.sync.dma_start(out=t[64:128, :], in_=bv_d[h].unsqueeze(1))
        bv_sb.append(t)
        t = atw.tile([128, 49], f32, tag=f"dqw{h}", name=f"dqw{h}")
        nc.sync.dma_start(out=t, in_=dwqw_d[h])
        dq_w.append(t)
        t = atw.tile([128, 1], f32, tag=f"dqb{h}", name=f"dqb{h}")
        nc.sync.dma_start(out=t, in_=dwqb_d[h].unsqueeze(1))
        dq_b.append(t)
        t = atw.tile([128, 392], f32, tag=f"ab{h}", name=f"ab{h}")
        nc.vector.memset(t[:], 0.0)
        nc.sync.dma_start(out=t[0:49, :], in_=ab_d[h])
        nc.sync.dma_start(out=t[64:113, :], in_=ab_d[h])
        ab_sb.append(t)
    # softmax helpers for pair-batched layout (imgs 2j/2j+1 at rows 0/64)
    ones2 = atw.tile([128, 2], bf16, tag="ones2", name="ones2")
    nc.vector.memset(ones2, 0.0)
    nc.vector.memset(ones2[0:49, 0:1], 1.0)
    nc.vector.memset(ones2[64:113, 1:2], 1.0)
    sel2 = atw.tile([2, 128], f32, tag="sel2", name="sel2")
    nc.vector.memset(sel2[0:1, :], 0.0)
    nc.vector.memset(sel2[0:1, 0:64], 1.0)
    sel2r1 = atw.tile([1, 128], f32, tag="sel2r1", name="sel2r1")
    nc.vector.memset(sel2r1[:], 0.0)
    nc.vector.memset(sel2r1[0:1, 64:128], 1.0)
    nc.sync.dma_start(out=sel2[1:2, :], in_=sel2r1[:])

    def prow(i):   # partition block for sp/vt/ein chain
        return 64 * (i % 2)

    def pcol(i):   # col block index for sp
        return i // 2

    def qrow(i):   # partition block for k/q'
        return 32 * (i % 4)

    def qcol(i):
        return i // 4

    with tc.tile_pool(name="sp", bufs=2) as spp, \
         tc.tile_pool(name="spx", bufs=2) as spxp, \
         tc.tile_pool(name="kqt", bufs=3) as kqtp, \
         tc.tile_pool(name="ksb", bufs=2) as ksbp, \
         tc.tile_pool(name="vt", bufs=2) as vtp, \
         tc.tile_pool(name="qs", bufs=2) as qsp, \
         tc.tile_pool(name="qpad", bufs=2) as qpp, \
         tc.tile_pool(name="att", bufs=3) as attp, \
         tc.tile_pool(name="spbf", bufs=2) as spbfp, \
         tc.tile_pool(name="aps", bufs=1, space="PSUM") as aps:

        sp_all = None
        sp_bf = None
        for h in range(NH):
            if h == 0:
                sp_all = spp.tile([128, NI // 2, POS], f32, tag="sp", name="sp0")
                sp_bf = spbfp.tile([128, NI // 2, POS], bf16, tag="spbf",
                                   name="spbf0")
                for img in range(NI):
                    nc.gpsimd.dma_start(
                        out=sp_all[prow(img):prow(img) + 64, pcol(img), :],
                        in_=x2wm[0][0:64, img, :])
                for j in range(4):
                    for n2 in range(2):
                        nc.vector.tensor_copy(
                            sp_bf[:, j, 392 * n2:392 * (n2 + 1)],
                            sp_all[:, j, 392 * n2:392 * (n2 + 1)])
            spn = spn_bf = None
            if h < NH - 1:
                spn = spp.tile([128, NI // 2, POS], f32, tag="sp", name=f"sp{h + 1}")
                spn_bf = spbfp.tile([128, NI // 2, POS], bf16, tag="spbf",
                                    name=f"spbf{h + 1}")
                spx = spxp.tile([128, NI // 2, POS], bf16, tag="spx", name=f"spx{h}")
                c_next, half_next = (h + 1) // 2, (h + 1) % 2
                for img in range(NI):
                    nc.gpsimd.dma_start(
                        out=spx[prow(img):prow(img) + 64, pcol(img), :],
                        in_=x2wm[c_next][64 * half_next:64 * half_next + 64,
                                         img, :])

            k_pk = ksbp.tile([128, 2, POS], bf16, tag="k", name=f"k{h}")
            vt_pk = vtp.tile([128, (NI // 2) * NW * D], bf16, tag="vt", name=f"vt{h}")
            qstack = qsp.tile([128, POS], bf16, tag="qstack", name=f"qstack{h}")
            qp_pk = qsp.tile([128, 2, POS], bf16, tag="qp", name=f"qp{h}")

            # ---- B: kq + v projections, per image pair ----
            for j in range(4):
                kqt = kqtp.tile([128, POS], bf16, tag="kqt", name=f"kqt{h}_{j}")
                for n2 in range(2):
                    pkq = aps.tile([128, 392], f32, tag="pkq", bufs=1,
                                   name=f"pkq{h}_{j}_{n2}")
                    pvt = aps.tile([128, 512], f32, tag="pvt", bufs=1,
                                   name=f"pvt{h}_{j}_{n2}")
                    for i01 in range(2):
                        p_ = 64 * i01
                        nc.tensor.matmul(pkq[p_:p_ + 2 * KD, :],
                                         wkq_sb[h][p_:p_ + 64, :],
                                         sp_bf[p_:p_ + 64, j,
                                               392 * n2:392 * (n2 + 1)],
                                         start=True, stop=True,
                                         tile_position=(p_, p_))
                        for w in range(8):
                            wg = 8 * n2 + w
                            nc.tensor.matmul(pvt[p_:p_ + WN, 64 * w:64 * (w + 1)],
                                             sp_bf[p_:p_ + 64, j,
                                                   WN * wg:WN * (wg + 1)],
                                             wv_sb[h][p_:p_ + 64, :],
                                             start=True, stop=True,
                                             tile_position=(p_, p_))
                    nc.scalar.activation(kqt[:, 392 * n2:392 * (n2 + 1)], pkq[:],
                                         AF.Identity, bias=bkq_sb[h][:, 0:1])
                    nc.scalar.activation(
                        vt_pk[:, 1024 * j + 512 * n2:1024 * j + 512 * (n2 + 1)],
                        pvt[:], AF.Copy)
                for i01 in range(2):
                    img = 2 * j + i01
                    q_ = qrow(img)
                    nc.sync.dma_start(
                        out=k_pk[q_:q_ + KD, qcol(img), :],
                        in_=kqt[64 * i01:64 * i01 + KD, :])
                    nc.sync.dma_start(
                        out=qstack[KD * img:KD * (img + 1), :],
                        in_=kqt[64 * i01 + KD:64 * i01 + 2 * KD, :])

            # ---- C: depthwise conv on stacked q (two guttered half-grids) ----
            kk = KS[h]
            p = kk // 2
            CW = 7 + p                   # cell stride
            HH = 2 * CW + p              # half-grid rows
            SW = 4 * CW + p              # grid cols
            SP = SW + (SW % 2)           # pad col stride to even (bf16 align)
            Lh, Lw = HH - 2 * p, SW - 2 * p
            qsv = qstack[:].rearrange("q (n h w) -> q n h w", n=NW, h=7)
            qflat = qsp.tile([128, NW, 49], bf16, tag="qflat", name=f"qflat{h}")
            for n2 in range(2):
                G = qpp.tile([128, 23, 44], bf16, tag="qpad",
                             name=f"qpad{h}_{n2}", padded_shape=None)
                nc.vector.memset(G[:, 0:HH, 0:SP], 0.0)
                for a2 in range(2):
                    nc.vector.tensor_copy(
                        cell_ap(G, p + CW * a2, p, CW),
                        qsv[:, 8 * n2 + 4 * a2:8 * n2 + 4 * a2 + 4])
                GA = qpp.tile([128, 23, 44], bf16, tag="qacc",
                              name=f"qacc{h}_{n2}")
                nc.vector.memset(GA[:, p:p + Lh, 0:44], 0.0)
                for t, (dy, dx) in enumerate(_dw_taps(kk)):
                    srcap = G[:, dy:dy + Lh, dx:dx + Lw]
                    dstap = GA[:, p:p + Lh, p:p + Lw]
                    nc.vector.scalar_tensor_tensor(dstap, srcap,
                                                   dq_w[h][:, t:t + 1],
                                                   dstap, AO.mult, AO.add)
                for a2 in range(2):
                    nc.vector.tensor_copy(
                        qflat[:, 8 * n2 + 4 * a2:8 * n2 + 4 * a2 + 4]
                        .rearrange("q n (h w) -> q n h w", h=7),
                        cell_ap(GA, p + CW * a2, p, CW))
                qfb = qsp.tile([128, 392], bf16, tag="qfb",
                               name=f"qfb{h}_{n2}")
                nc.scalar.activation(
                    qfb[:], qflat[:, 8 * n2:8 * n2 + 8, :]
                    .rearrange("q n s -> q (n s)"),
                    AF.Identity, bias=dq_b[h][:, 0:1])
                for img in range(NI):
                    nc.sync.dma_start(
                        out=qp_pk[qrow(img):qrow(img) + KD, qcol(img),
                                  392 * n2:392 * (n2 + 1)],
                        in_=qfb[KD * img:KD * (img + 1), :])

            # ---- D: attention per (pair, half) ----
            cy, hy = h // 2, h % 2
            for j in range(4):
                for n2 in range(2):
                    co = 392 * n2
                    pa = aps.tile([128, 392], f32, tag="pa", bufs=2,
                                  name=f"pa{h}_{j}_{n2}")
                    for i01 in range(2):
                        img = 2 * j + i01
                        p_, q_ = 64 * i01, qrow(img)
                        for w in range(8):
                            wg = 8 * n2 + w
                            nc.tensor.matmul(
                                pa[p_:p_ + WN, WN * w:WN * (w + 1)],
                                k_pk[q_:q_ + KD, qcol(img), WN * wg:WN * (wg + 1)],
                                qp_pk[q_:q_ + KD, qcol(img), WN * wg:WN * (wg + 1)],
                                start=True, stop=True, tile_position=(q_, p_))
                    ein = attp.tile([128, 392], bf16, tag="ein",
                                    name=f"ein{h}_{j}_{n2}")
                    nc.vector.scalar_tensor_tensor(ein[:], pa[:], 20.0,
                                                   ab_sb[h][:], AO.min, AO.add)
                    eexp = attp.tile([128, 392], bf16, tag="eexp",
                                     name=f"eexp{h}_{j}_{n2}")
                    nc.scalar.activation(eexp[:], ein[:], AF.Exp)
                    ps1 = aps.tile([2, 392], f32, tag="ps1", bufs=1,
                                   name=f"ps1{h}_{j}_{n2}")
                    nc.tensor.matmul(ps1[:], ones2[:], eexp[:],
                                     start=True, stop=True)
                    rs = attp.tile([2, 392], f32, tag="rs", name=f"rs{h}_{j}_{n2}")
                    nc.vector.reciprocal_approx_fast(rs[:], ps1[:])
                    pbc = aps.tile([128, 392], f32, tag="pbc", bufs=1,
                                   name=f"pbc{h}_{j}_{n2}")
                    nc.tensor.matmul(pbc[:], sel2[:], rs[:],
                                     start=True, stop=True)
                    bc = attp.tile([128, 392], bf16, tag="bc",
                                   name=f"bc{h}_{j}_{n2}")
                    nc.scalar.activation(bc[:], pbc[:], AF.Copy)
                    pav = aps.tile([128, 392], f32, tag="pav", bufs=2,
                                   name=f"pav{h}_{j}_{n2}")
                    for i01 in range(2):
                        p_ = 64 * i01
                        for w in range(8):
                            wg = 8 * n2 + w
                            nc.tensor.matmul(
                                pav[p_:p_ + D, WN * w:WN * (w + 1)],
                                vt_pk[p_:p_ + WN,
                                      1024 * j + 64 * wg:1024 * j + 64 * (wg + 1)],
                                eexp[p_:p_ + WN, WN * w:WN * (w + 1)],
                                start=True, stop=True, tile_position=(p_, p_))
                    spo = attp.tile([128, 392], bf16, tag="spo",
                                    name=f"spo{h}_{j}_{n2}")
                    nc.vector.tensor_tensor(spo[:], pav[:], bc[:], AO.mult)
                    for i01 in range(2):
                        img = 2 * j + i01
                        nc.gpsimd.dma_start(
                            out=y_dram[cy, 64 * hy:64 * hy + 64, img, co:co + 392],
                            in_=spo[64 * i01:64 * i01 + 64, :])
                    if h < NH - 1:
                        nc.vector.scalar_tensor_tensor(
                            spn[:, j, co:co + 392], spo[:],
                            bv_sb[h][:, 0:1], spx[:, j, co:co + 392],
                            AO.add, AO.add)
                        nc.vector.tensor_copy(
                            spn_bf[:, j, co:co + 392],
                            spn[:, j, co:co + 392])
            sp_all = spn
            sp_bf = spn_bf

    atw_cm.__exit__(None, None, None)

    # -------- P4+P5+P6: y hswish + proj + dw1 + MLP1 + out, pipelined ------
    with tc.tile_pool(name="pjw", bufs=1) as pjw, \
         tc.tile_pool(name="hyp", bufs=3) as hyp, \
         tc.tile_pool(name="x2r", bufs=2) as x2rp, \
         tc.tile_pool(name="x3p", bufs=2) as x3p, \
         tc.tile_pool(name="dwp1", bufs=2) as padp1, \
         tc.tile_pool(name="dwa1", bufs=3) as accp1, \
         tc.tile_pool(name="x4p", bufs=2) as x4p, \
         tc.tile_pool(name="m1h", bufs=2) as hp1, \
         tc.tile_pool(name="m1r", bufs=3) as rp1, \
         tc.tile_pool(name="m1z", bufs=3) as zp1, \
         tc.tile_pool(name="o5", bufs=3) as o5p, \
         tc.tile_pool(name="ppp", bufs=2, space="PSUM") as ppp, \
         tc.tile_pool(name="m1ps", bufs=4, space="PSUM") as psp1, \
         tc.tile_pool(name="m1po", bufs=2, space="PSUM") as pop1:
        pj_sb = []
        for k in range(4):
            w = pjw.tile([128, ED], bf16, tag=f"pj{k}")
            nc.sync.dma_start(out=w, in_=projT_d[128 * k:128 * (k + 1), :])
            pj_sb.append(w)
        pjb_sb, yb_sb, yb05_sb = [], [], []
        for m in range(4):
            b = pjw.tile([128, 1], f32, tag=f"pjb{m}")
            nc.sync.dma_start(out=b, in_=projb_d[128 * m:128 * (m + 1)].unsqueeze(1))
            pjb_sb.append(b)
            b = pjw.tile([128, 1], f32, tag=f"ybt{m}")
            nc.sync.dma_start(out=b, in_=yb_d[128 * m:128 * (m + 1)].unsqueeze(1))
            yb_sb.append(b)
            b05 = pjw.tile([128, 1], f32, tag=f"yb05{m}")
            nc.vector.tensor_scalar(b05[:], yb_sb[m][:], 1.0 / 6.0, 0.5,
                                    AO.mult, AO.add)
            yb05_sb.append(b05)
        acts2 = pjw.tile([128, 1], f32, tag="acts2")
        nc.vector.memset(acts2, 1.0 / 6.0)
        w1sb_, b1sb_ = [], []
        for c in range(4):
            w = pjw.tile([128, 9], f32, tag=f"dw1w{c}")
            nc.sync.dma_start(out=w, in_=dw1w_d[c])
            b = pjw.tile([128, 1], f32, tag=f"dw1b{c}")
            nc.sync.dma_start(out=b, in_=dw1b_d[c].unsqueeze(1))
            w1sb_.append(w)
            b1sb_.append(b)
        w1m = []
        for k in range(4):
            w = pjw.tile([128, 2 * ED], bf16, tag=f"m1w1_{k}")
            nc.sync.dma_start(out=w, in_=w1T1_d[128 * k:128 * (k + 1), :])
            w1m.append(w)
        w2m = []
        for k in range(8):
            w = pjw.tile([128, ED], bf16, tag=f"m1w2_{k}")
            nc.sync.dma_start(out=w, in_=w2T1_d[128 * k:128 * (k + 1), :])
            w2m.append(w)
        b1row1 = pjw.tile([1, 2 * ED], bf16, tag="b1row1")
        nc.sync.dma_start(out=b1row1, in_=b1f1_d.unsqueeze(0))
        ones392b = pjw.tile([1, 392], bf16, tag="ones392b")
        nc.vector.memset(ones392b, 1.0)
        acth1 = pjw.tile([128, 1], f32, tag="acth1")
        nc.vector.memset(acth1, 0.5)
        b2m = []
        for m in range(4):
            b = pjw.tile([128, 1], f32, tag=f"m1b2_{m}")
            nc.sync.dma_start(out=b, in_=b2f1_d[128 * m:128 * (m + 1)].unsqueeze(1))
            b2m.append(b)

        for j in range(4):
            x3s = {}
            for i01 in range(2):
                img = 2 * j + i01
                hys = []
                for c in range(4):
                    yt = hyp.tile([128, POS], bf16, tag="yt", name=f"yt{c}_{img}")
                    nc.sync.dma_start(out=yt, in_=y_dram[c, :, img, :])
                    z = hyp.tile([128, POS], bf16, tag="z", name=f"z{c}_{img}")
                    nc.scalar.activation(z[:], yt[:], AF.Identity,
                                         bias=yb_sb[c][:, 0:1])
                    hy = hyp.tile([128, POS], bf16, tag=f"hy{c}",
                                  name=f"hy{c}_{img}")
                    for n2 in range(2):
                        r = rp1.tile([128, 392], bf16, tag="pr")
                        nc.scalar.activation(r[:], yt[:, 392 * n2:392 * (n2 + 1)],
                                             AF.Relu, scale=acts2[:, 0:1],
                                             bias=yb05_sb[c][:, 0:1])
                        nc.vector.scalar_tensor_tensor(
                            hy[:, 392 * n2:392 * (n2 + 1)], r[:], 1.0,
                            z[:, 392 * n2:392 * (n2 + 1)], AO.min, AO.mult)
                    hys.append(hy)
                x2rb = [x2wm[c][:, img, :] for c in range(4)]
                for mo in range(4):
                    x3wm = x3p.tile([128, NW, 49], bf16, tag=f"x3{mo}",
                                    name=f"x3{mo}_{img}")
                    x3s[(mo, i01)] = x3wm
                    for n2 in range(2):
                        pp = ppp.tile([128, 392], f32, tag="pp")
                        for k in range(4):
                            nc.tensor.matmul(pp[:],
                                             pj_sb[k][:, 128 * mo:128 * (mo + 1)],
                                             hys[k][:, 392 * n2:392 * (n2 + 1)],
                                             start=(k == 0), stop=False)
                        nc.tensor.matmul(pp[:], ident_sb[:],
                                         x2rb[mo][:, 392 * n2:392 * (n2 + 1)],
                                         start=False, stop=True)
                        nc.scalar.activation(
                            x3wm[:].rearrange("p a w -> p (a w)")
                            [:, 392 * n2:392 * (n2 + 1)], pp[:],
                            AF.Identity, bias=pjb_sb[mo][:, 0:1])

            # dw1 on the pair: x3 wm -> DRAM -> spatial reload, clipped taps
            x4s = {}
            for c in range(4):
                for i01 in range(2):
                    im = 2 * j + i01
                    x3sp = padp1.tile([128, 28, 28], bf16, tag="x3sp",
                                      name=f"x3sp{c}_{im}")
                    x3wmv = x3s[(c, i01)][:].rearrange(
                        "p nw s -> p (nw s)").rearrange(
                        "p (b a h w) -> p b a h w", b=4, a=4, h=7)
                    for b in range(4):
                        nc.gpsimd.tensor_copy(
                            x3sp[:, :, 7 * b:7 * (b + 1)]
                            .rearrange("p (a h) w -> p a h w", a=4),
                            x3wmv[:, b])
                    acc = accp1.tile([128, 28, 28], f32, tag="acc1",
                                     name=f"acc1_{c}_{im}")
                    first = True
                    for t, (dy, dx) in _dw_taps3_sorted():
                        r0, r1 = max(0, dy - 1), 28 + min(0, dy - 1)
                        c0, c1 = max(0, dx - 1), 28 + min(0, dx - 1)
                        src = x3sp[:, r0:r1, c0:c1]
                        dst = acc[:, max(0, 1 - dy):max(0, 1 - dy) + (r1 - r0),
                                  max(0, 1 - dx):max(0, 1 - dx) + (c1 - c0)]
                        if first:
                            # acc = w_center*x3 + x3 (residual folded in)
                            nc.vector.scalar_tensor_tensor(
                                dst, src, w1sb_[c][:, t:t + 1], x3sp[:],
                                AO.mult, AO.add)
                            first = False
                        else:
                            nc.vector.scalar_tensor_tensor(
                                dst, src, w1sb_[c][:, t:t + 1], dst,
                                AO.mult, AO.add)
                    x4 = x4p.tile([128, 28, 28], bf16, tag=f"x4_{c}",
                                  name=f"x4_{c}_{im}")
                    nc.vector.scalar_tensor_tensor(
                        x4[:], acc[:], b1sb_[c][:, 0:1], acc[:],
                        AO.add, AO.bypass)
                    x4s[(c, i01)] = x4

            for i01 in range(2):
                img = 2 * j + i01
                x4f = [x4s[(c, i01)][:].rearrange("p h w -> p (h w)")
                       for c in range(4)]
                hs = [hp1.tile([128, POS], bf16, tag=f"g{m}", name=f"g{m}_{img}")
                      for m in range(8)]
                for m in range(8):
                    for n2 in range(2):
                        ph = psp1.tile([128, 392], f32, tag="ph1")
                        for k in range(4):
                            nc.tensor.matmul(
                                ph[:], w1m[k][:, 128 * m:128 * (m + 1)],
                                x4f[k][:, 392 * n2:392 * (n2 + 1)],
                                start=(k == 0), stop=False)
                        nc.tensor.matmul(
                            ph[:], b1row1[:, 128 * m:128 * (m + 1)],
                            ones392b[:], start=False, stop=True)
                        r = rp1.tile([128, 392], bf16, tag="r1")
                        nc.scalar.activation(r[:], ph[:], AF.Relu,
                                             scale=acts2[:, 0:1],
                                             bias=acth1[:, 0:1])
                        nc.vector.scalar_tensor_tensor(
                            hs[m][:, 392 * n2:392 * (n2 + 1)], r[:], 1.0,
                            ph[:], AO.min, AO.mult)
                for mo in range(4):
                    for n2 in range(2):
                        po = pop1.tile([128, 392], f32, tag="po1")
                        for k in range(8):
                            nc.tensor.matmul(
                                po[:], w2m[k][:, 128 * mo:128 * (mo + 1)],
                                hs[k][:, 392 * n2:392 * (n2 + 1)],
                                start=(k == 0), stop=False)
                        nc.tensor.matmul(
                            po[:], ident_sb[:],
                            x4f[mo][:, 392 * n2:392 * (n2 + 1)],
                            start=False, stop=True)
                        x5 = o5p.tile([128, 392], f32, tag="x5",
                                      name=f"x5_{mo}_{img}_{n2}")
                        nc.scalar.activation(x5[:], po[:], AF.Identity,
                                             bias=b2m[mo][:, 0:1])
                        nc.sync.dma_start(
                            out=out_d[img, 128 * mo:128 * (mo + 1),
                                      392 * n2:392 * (n2 + 1)],
                            in_=x5[:])

    misc_cm.__exit__(None, None, None)
    x2wm_cm.__exit__(None, None, None)
    dram_cm.__exit__(None, None, None)


# ---------------------------------------------------------------------------
# host-side input preprocessing
# ---------------------------------------------------------------------------

def prep_weights(inp):
    def taps(w):  # [C,1,k,k] -> [C, k*k]
        return w.reshape(w.shape[0], -1).astype(np.float32)

    m = {}
    m["dw0w"] = taps(inp["dw0_w"]).reshape(4, 128, 9)
    m["dw0b"] = inp["dw0_b"].reshape(4, 128).astype(np.float32)
    m["w1T0"] = np.ascontiguousarray(inp["ffn0_w1"].T).astype(ml_dtypes.bfloat16)
    m["b1f0"] = inp["ffn0_b1"].astype(ml_dtypes.bfloat16)
    m["w2T0"] = np.ascontiguousarray(inp["ffn0_w2"].T).astype(ml_dtypes.bfloat16)
    m["b2f0"] = inp["ffn0_b2"].astype(np.float32)

    qkv_w, qkv_b = inp["qkv_w"], inp["qkv_b"]
    # reorder rows: k(16:32) first, then q(0:16); v separate
    wkqT = np.empty((NH, D, 2 * KD), np.float32)
    bkq = np.empty((NH, 2 * KD), np.float32)
    wvT = np.empty((NH, D, D), np.float32)
    bv = np.empty((NH, D), np.float32)
    for h in range(NH):
        W = qkv_w[h]  # [96, 64]
        wkqT[h, :, 0:KD] = W[KD:2 * KD].T
        wkqT[h, :, KD:2 * KD] = W[0:KD].T
        bkq[h, 0:KD] = qkv_b[h, KD:2 * KD]
        bkq[h, KD:2 * KD] = qkv_b[h, 0:KD]
        wvT[h] = W[2 * KD:].T
        bv[h] = qkv_b[h, 2 * KD:]
    m["wkqT"] = wkqT.astype(ml_dtypes.bfloat16)
    m["bkq"] = bkq
    m["wvT"] = wvT.astype(ml_dtypes.bfloat16)
    m["bv"] = bv

    dwq_ws = [inp["dwq_w7"], inp["dwq_w5"]] + [inp["dwq_w3"][i] for i in range(6)]
    dwq_bs = [inp["dwq_b7"], inp["dwq_b5"]] + [inp["dwq_b3"][i] for i in range(6)]
    dwqw = np.zeros((NH, 128, 49), np.float32)
    dwqb = np.zeros((NH, 128), np.float32)
    for h in range(NH):
        t = taps(dwq_ws[h]) * SCALE          # [16, k*k]
        nt = t.shape[1]
        for i in range(NI):
            dwqw[h, KD * i:KD * (i + 1), :nt] = t
            dwqb[h, KD * i:KD * (i + 1)] = dwq_bs[h] * SCALE
    m["dwqw"] = dwqw
    m["dwqb"] = dwqb

    ab = inp["attn_bias"][:, BIAS_IDX]       # [NH, 49, 49]
    m["ab"] = np.tile(ab, (1, 1, 8)).astype(np.float32)  # [NH, 49, 392]

    m["projT"] = np.ascontiguousarray(inp["proj_w"].T).astype(ml_dtypes.bfloat16)
    m["projb"] = inp["proj_b"].astype(np.float32)
    m["yb"] = bv.reshape(ED).astype(np.float32)

    m["dw1w"] = taps(inp["dw1_w"]).reshape(4, 128, 9)
    m["dw1b"] = inp["dw1_b"].reshape(4, 128).astype(np.float32)
    m["w1T1"] = np.ascontiguousarray(inp["ffn1_w1"].T).astype(ml_dtypes.bfloat16)
    m["b1f1"] = inp["ffn1_b1"].astype(ml_dtypes.bfloat16)
    m["w2T1"] = np.ascontiguousarray(inp["ffn1_w2"].T).astype(ml_dtypes.bfloat16)
    m["b2f1"] = inp["ffn1_b2"].astype(np.float32)
    m["ident"] = np.eye(128, dtype=np.float32).astype(ml_dtypes.bfloat16)
    return m


@functools.lru_cache(maxsize=1)
def _cached_program():
    return build_program()


def _run(inputs, trace=False, **kw):
    nc = _cached_program()
    wm = prep_weights(inputs)
    x = np.asarray(inputs["x"], dtype=np.float32).reshape(64, ED, POS)
    in_maps = []
    for core in range(NCORES):
        im = dict(wm)
        im["x"] = np.ascontiguousarray(x[NI * core:NI * (core + 1)])
        in_maps.append(im)
    res = bass_utils.run_bass_kernel_spmd(nc, in_maps, list(range(NCORES)),
                                          trace=trace, **kw)
    out = np.concatenate([r["out"] for r in res.results], axis=0)
    return out.reshape(64, ED, RES, RES).astype(np.float32), res


def kernel(**inputs):
    out, _ = _run(inputs)
    return out

